# revision 20
# baseline (speedup 1.0000x reference)
"""Trainium2 Bass kernel for nn_Enet_81037442941606 (gnn_message_passing).

Computation (reference):
    g   = enc_out[batch_idx, tgt]                      # [N, D] gather
    h0  = batchnorm(g)  (training stats, biased var)   # [N, D]
    h1  = swish(h0 @ wt2_w.T + wt2_b)                  # [N, C]
    out = h1 @ A.T + h1   (A sparse, NNZ entries)      # [N, C]

Strategy (8 NeuronCores, tensor parallel over the class axis):
  * Each core owns a contiguous block of C/8 = 8192 classes: its wt2_w rows,
    its A rows (spmm output rows), and its output columns.
  * The h1^T shard exchange (bf16 AllGather, ~260us of CC ring time) is the
    critical resource.  It is split into 5 chunked collectives over class
    sub-ranges, each fired as soon as the main matmul finishes that
    sub-range, so the CC stream overlaps the matmul and most of the spmm.
  * The spmm runs in 4 passes keyed to AllGather-chunk completion: the host
    sorts each output row-block's contributions by source availability and
    packs them into 128-slot selection chunks; a chunk runs in the earliest
    pass whose ag_out prefix covers all its sources.  Partial psum results
    between passes are parked in SBUF (bf16) and merged back via identity
    matmuls.  Row gathers use the batched dma_gather instruction (int16
    indices, so sources address one of two 32768-row windows of ag_out;
    chunks are kept window-pure by the packing).
  * Host concatenates the 8 output shards and transposes back to [N, C].
"""

import numpy as np
import ml_dtypes

import concourse.bacc as bacc
import concourse.bass as bass
import concourse.mybir as mybir
import concourse.tile as tile
from concourse.bass_utils import run_bass_kernel_spmd
from concourse.masks import make_identity

# Problem sizes (hardcoded per contest rules).
B, S, D, C, N = 32, 128, 1024, 65536, 512
NNZ = 262144
EPS = 1e-5
NCORES = 8
CLOC = C // NCORES          # classes per core = 8192
NB = CLOC // 128            # 64 c-tiles (row-blocks) per core
NBH = NB // 2               # 32 c-tiles per half
ND = D // 128               # 8 contraction chunks
NT = N // 128               # 4 token tiles
P = 128

EX_DT = mybir.dt.bfloat16   # h1 exchange dtype
EX_NP = ml_dtypes.bfloat16
MM_DT = mybir.dt.bfloat16   # main-matmul operand dtype (W, h0^T)
MM_NP = ml_dtypes.bfloat16

# AllGather chunking (c-tiles per chunk) and spmm pass prefixes (c-tiles).
TCH = (8, 24, 24, 8)
CUMT = (0, 8, 32, 56, 64)
NAG = len(TCH)
PASS_PREFIX = (8, 32, 56, 64)       # pass j sources lie in tiles < this
NPASS = len(PASS_PREFIX)
NPASS_A = 2                         # passes sourcing window A (prefix <= 32)
HALF_ROWS = NBH * NCORES * P        # 32768: dma_gather int16 window size
WB = 4                      # W tiles per load group
GB = 4                      # sel-chunks per batched spmm gather
SB = 32                     # sel-chunks per sel-matrix DMA

_PROGRAM_CACHE = {}
TRACE = False          # set by test.py to capture an NTFF profile
LAST_RESULTS = None    # BassKernelResults of the last kernel() call


def _build_program(profile):
    """Build + compile the SPMD Bass program (identical on all 8 cores).

    profile = (chunks_a, chunks_b, passes):
      chunks_a[rb]/chunks_b[rb] = sel chunks sourcing window A (pass 0) /
      window B (passes 1-3) for row block rb; passes[rb] = per-chunk pass.
    """
    if profile in _PROGRAM_CACHE:
        return _PROGRAM_CACHE[profile]
    chunks_a, chunks_b, passes = profile
    chunks = tuple(a + b for a, b in zip(chunks_a, chunks_b))
    tot_ch = sum(chunks)

    # Global chunk order: (pass, rb, chunk_idx).
    order = sorted(
        (passes[rb][ci], rb, ci) for rb in range(NB) for ci in range(chunks[rb])
    )
    gidx_of = {(rb, ci): gi for gi, (_, rb, ci) in enumerate(order)}
    sessions = [[] for _ in range(NPASS)]
    first_pass = {}
    last_pass = {}
    for rb in range(NB):
        ps = passes[rb]
        first_pass[rb] = ps[0]
        last_pass[rb] = ps[-1]
        for j in sorted(set(ps)):
            cis = [ci for ci in range(chunks[rb]) if ps[ci] == j]
            gi0 = gidx_of[(rb, cis[0])]
            sessions[j].append((rb, gi0, gi0 + len(cis)))
    for j in range(NPASS):
        sessions[j].sort(key=lambda s: s[1])
    pass_start = [min((s[1] for s in sessions[j]), default=0)
                  for j in range(NPASS)]

    nc = bacc.Bacc("TRN2", target_bir_lowering=False, debug=False,
                   num_devices=NCORES)
    f32 = mybir.dt.float32
    i16 = mybir.dt.int16

    enc = nc.dram_tensor("enc", [B * S, D], f32, kind="ExternalInput")
    gidx = nc.dram_tensor("gidx", [P, NT * P // 16], i16, kind="ExternalInput")
    wt = nc.dram_tensor("wt", [NB, P, D], MM_DT, kind="ExternalInput")
    biasv = nc.dram_tensor("biasv", [P, NB], f32, kind="ExternalInput")
    sel = nc.dram_tensor("sel", [P, tot_ch * P], EX_DT, kind="ExternalInput")
    gidxs = nc.dram_tensor("gidxs", [P, tot_ch * 8], i16, kind="ExternalInput")
    outT = nc.dram_tensor("outT", [CLOC, N], f32, kind="ExternalOutput")

    ag_ins = [nc.dram_tensor(f"ag_in{k}", [TCH[k] * P, N], EX_DT)
              for k in range(NAG)]
    ag_out = nc.dram_tensor("ag_out", [C, N], EX_DT, addr_space="Shared")
    # ag_out row space is chunk-major: chunk k rows live at
    # 8*P*CUMT[k] + rr*P*TCH[k] + (l - P*CUMT[k]).  Host maps gather
    # indices to this layout, relative to the pass's window base.
    ag_in_vs = [t.ap().rearrange("(i p) n -> p i n", p=P) for t in ag_ins]
    outT_v = outT.ap().rearrange("(i p) n -> i p n", p=P)

    with tile.TileContext(nc) as tc:
        with (
            tc.tile_pool(name="persist", bufs=1) as persist,
        ):
            h0T = persist.tile([P, ND * N], MM_DT)      # [d%128, (dchunk, n)]
            h1T = persist.tile([P, NB * N], EX_DT)      # [c%128, (ctile, n)]
            partial = persist.tile([P, NB * N], EX_DT)  # spmm pass partials
            bias_t = persist.tile([P, NB], f32)
            gidxs_t = persist.tile([P, tot_ch * 8], i16)
            ident = persist.tile([P, P], f32)
            identb = persist.tile([P, P], EX_DT)
            mean_s = persist.tile([P, ND], f32)
            rstd_s = persist.tile([P, ND], f32)
            gidx_t = persist.tile([P, NT * P // 16], i16)

            nc.sync.dma_start(out=gidx_t[:], in_=gidx[:])
            make_identity(nc, ident[:])
            nc.vector.tensor_copy(out=identb[:], in_=ident[:])
            nc.sync.dma_start(out=bias_t[:], in_=biasv[:])
            nc.sync.dma_start(out=gidxs_t[:], in_=gidxs[:])

            # ---------------- Phase A: gather + batchnorm + h0^T -----------
            with (
                tc.tile_pool(name="phA", bufs=1) as phA,
                tc.tile_pool(name="psA", bufs=4, space="PSUM") as psA,
            ):
                g_all = phA.tile([P, NT * D], f32, tag="g")
                nc.gpsimd.dma_gather(
                    out_ap=g_all[:].rearrange("p (b e) -> p b e", e=D),
                    in_ap=enc[:],
                    idxs_ap=gidx_t[:],
                    num_idxs=NT * P,
                    num_idxs_reg=NT * P,
                    elem_size=D,
                )

                # Raw transpose g -> h0T (tokens on the free axis), d-chunk
                # major so per-chunk batch stats chase the transposes.
                sum_s = phA.tile([P, ND], f32, tag="sums")
                sq_s = phA.tile([P, ND], f32, tag="sqs")
                scr = phA.tile([P, N], f32, tag="scr")
                for i in range(ND):
                    for j in range(NT):
                        tp = psA.tile([P, P], f32, space="PSUM", tag="tp")
                        nc.tensor.transpose(
                            tp[:], g_all[:, j * D + i * P: j * D + (i + 1) * P],
                            ident[:])
                        nc.vector.tensor_copy(
                            out=h0T[:, i * N + j * P: i * N + (j + 1) * P],
                            in_=tp[:])
                    nc.scalar.activation(
                        scr[:], h0T[:, i * N:(i + 1) * N],
                        mybir.ActivationFunctionType.Copy,
                        accum_out=sum_s[:, i:i + 1])
                    nc.scalar.activation(
                        scr[:], h0T[:, i * N:(i + 1) * N],
                        mybir.ActivationFunctionType.Square,
                        accum_out=sq_s[:, i:i + 1])

                ex2_s = phA.tile([P, ND], f32, tag="ex2")
                var_s = phA.tile([P, ND], f32, tag="var")
                nc.scalar.mul(mean_s[:], sum_s[:], 1.0 / N)
                nc.scalar.mul(ex2_s[:], sq_s[:], 1.0 / N)
                nc.vector.tensor_tensor(
                    out=var_s[:], in0=mean_s[:], in1=mean_s[:],
                    op=mybir.AluOpType.mult)
                nc.vector.tensor_tensor(
                    out=var_s[:], in0=ex2_s[:], in1=var_s[:],
                    op=mybir.AluOpType.subtract)
                sd_s = phA.tile([P, ND], f32, tag="sd")
                epsb = phA.tile([P, 1], f32, tag="epsb")
                nc.vector.memset(epsb[:], EPS)
                nc.scalar.activation(
                    sd_s[:], var_s[:], mybir.ActivationFunctionType.Sqrt,
                    bias=epsb[:, :1], scale=1.0)
                nc.vector.reciprocal(rstd_s[:], sd_s[:])

                for i in range(ND):
                    nc.vector.tensor_scalar(
                        out=h0T[:, i * N:(i + 1) * N],
                        in0=h0T[:, i * N:(i + 1) * N],
                        scalar1=mean_s[:, i:i + 1],
                        scalar2=rstd_s[:, i:i + 1],
                        op0=mybir.AluOpType.subtract,
                        op1=mybir.AluOpType.mult,
                    )

            # ---------------- Phases B + D interleaved ---------------------
            # B: h1^T = swish(W h0^T + b), in AG-chunk order; each chunked
            # AllGather fires as soon as its c-tile range is in ag_in[k].
            # D: spmm passes are emitted between B chunks so the gpsimd
            # engine can start pass-j gathers as soon as AG chunk j lands,
            # without blocking later AG triggers (in-order engines).
            # Finals (which read h1T residuals) only occur in passes >= 2,
            # after all of B has been emitted.
            assert all(ps[-1] >= NPASS_A for ps in passes)
            wt_b = wt.ap().rearrange("(a b) p d -> a b p d", b=WB)
            with (
                tc.tile_pool(name="phB", bufs=2) as phB,
                tc.tile_pool(name="psB", bufs=4, space="PSUM") as psB,
                tc.tile_pool(name="ctp", bufs=7) as ctp,
                tc.tile_pool(name="selp", bufs=2) as selp,
                tc.tile_pool(name="otp", bufs=3) as otp,
                tc.tile_pool(name="psD", bufs=4, space="PSUM") as psD,
            ):
                ct_tiles = {}
                sel_tiles = {}

                def emit_b_chunk(k):
                    for a in range(CUMT[k] // WB, CUMT[k + 1] // WB):
                        wt_a = phB.tile([P, WB * D], MM_DT, tag="wt")
                        nc.sync.dma_start(
                            out=wt_a[:].rearrange("p (b d) -> p b d", b=WB),
                            in_=wt_b[a].rearrange("b p d -> p b d"))
                        for bsub in range(WB):
                            i = a * WB + bsub
                            h1ps = psB.tile([P, N], f32, space="PSUM",
                                            tag="h1ps")
                            for kk in range(ND):
                                nc.tensor.matmul(
                                    out=h1ps[:],
                                    lhsT=wt_a[:, bsub * D + kk * P:
                                              bsub * D + (kk + 1) * P],
                                    rhs=h0T[:, kk * N:(kk + 1) * N],
                                    start=(kk == 0), stop=(kk == ND - 1),
                                )
                            nc.scalar.activation(
                                h1T[:, i * N:(i + 1) * N], h1ps[:],
                                mybir.ActivationFunctionType.Silu,
                                bias=bias_t[:, i:i + 1], scale=1.0)
                        g0 = a * WB
                        nc.sync.dma_start(
                            out=ag_in_vs[k][:, g0 - CUMT[k]:
                                            g0 - CUMT[k] + WB, :],
                            in_=h1T[:, g0 * N:(g0 + WB) * N].rearrange(
                                "p (i n) -> p i n", n=N))
                    r0, r1 = NCORES * P * CUMT[k], NCORES * P * CUMT[k + 1]
                    nc.gpsimd.collective_compute(
                        "AllGather",
                        mybir.AluOpType.bypass,
                        replica_groups=[list(range(NCORES))],
                        ins=[ag_ins[k][:].opt()],
                        outs=[ag_out[r0:r1, :].opt()],
                    )

                def ensure_ct(j, b, pend):
                    key = (j, b)
                    if key in ct_tiles:
                        return ct_tiles[key]
                    a0 = pass_start[j] + b * GB
                    a1 = min(a0 + GB, pend)
                    m = a1 - a0
                    t = ctp.tile([P, GB * N], EX_DT, tag="ct")
                    base = 0 if j < NPASS_A else HALF_ROWS
                    ext = NCORES * P * PASS_PREFIX[j] - base
                    nc.gpsimd.dma_gather(
                        out_ap=t[:, :m * N].rearrange("p (b n) -> p b n", n=N),
                        in_ap=ag_out[base:base + ext, :],
                        idxs_ap=gidxs_t[:, a0 * 8:a1 * 8],
                        num_idxs=m * P,
                        num_idxs_reg=m * P,
                        elem_size=N,
                    )
                    ct_tiles[key] = t
                    return t

                def ensure_sel(b):
                    if b in sel_tiles:
                        return sel_tiles[b]
                    a0 = b * SB
                    a1 = min(a0 + SB, tot_ch)
                    m = a1 - a0
                    t = selp.tile([P, SB * P], EX_DT, tag="sel")
                    nc.sync.dma_start(
                        out=t[:, :m * P], in_=sel[:, a0 * P:a1 * P])
                    sel_tiles[b] = t
                    return t

                def emit_d_pass(j):
                    if not sessions[j]:
                        return
                    pend = max(s[2] for s in sessions[j])
                    for rb, gi0, gi1 in sessions[j]:
                        has_prev = j > first_pass[rb]
                        is_last = j == last_pass[rb]
                        acc = psD.tile([P, N], f32, space="PSUM", tag="acc")
                        for gi in range(gi0, gi1):
                            ctb = (gi - pass_start[j]) // GB
                            cto = gi - pass_start[j] - ctb * GB
                            ct = ensure_ct(j, ctb, pend)
                            slb, slo = gi // SB, gi % SB
                            sl = ensure_sel(slb)
                            last_mm = (gi == gi1 - 1) and not has_prev
                            nc.tensor.matmul(
                                out=acc[:],
                                lhsT=sl[:, slo * P:(slo + 1) * P],
                                rhs=ct[:, cto * N:(cto + 1) * N],
                                start=(gi == gi0), stop=last_mm,
                            )
                        if has_prev:
                            nc.tensor.matmul(
                                out=acc[:],
                                lhsT=identb[:],
                                rhs=partial[:, rb * N:(rb + 1) * N],
                                start=False, stop=True,
                            )
                        if is_last:
                            o_t = otp.tile([P, N], f32, tag="ot")
                            nc.vector.tensor_tensor(
                                out=o_t[:], in0=acc[:],
                                in1=h1T[:, rb * N:(rb + 1) * N],
                                op=mybir.AluOpType.add)
                            nc.sync.dma_start(out=outT_v[rb], in_=o_t[:])
                        else:
                            nc.scalar.activation(
                                partial[:, rb * N:(rb + 1) * N], acc[:],
                                mybir.ActivationFunctionType.Copy)

                for k in range(NAG):
                    emit_b_chunk(k)
                for j in range(NPASS):
                    emit_d_pass(j)

    nc.compile()
    _PROGRAM_CACHE[profile] = nc
    return nc


def _wrap16(idx_flat):
    """Layout flat gather indices for dma_gather: idx i -> [i%16, i//16],
    replicated across the 8 groups of 16 partitions."""
    n = len(idx_flat)
    assert n % 16 == 0
    a = np.asarray(idx_flat, dtype=np.int16).reshape(n // 16, 16).T  # [16, n/16]
    return np.tile(a, (8, 1))                                        # [128, n/16]


def _prep_host(enc_out, wt2_w, wt2_b, A_values, batch_idx, tgt, A_indices):
    """Shard inputs + restructure the sparse matrix for the device program."""
    enc_flat = np.ascontiguousarray(
        np.asarray(enc_out, dtype=np.float32).reshape(B * S, D))
    flat_idx = (np.asarray(batch_idx, dtype=np.int64) * S
                + np.asarray(tgt, dtype=np.int64))
    gidx_host = np.ascontiguousarray(_wrap16(flat_idx))

    wt2_w = np.asarray(wt2_w, dtype=np.float32)
    wt2_b = np.asarray(wt2_b, dtype=np.float32)
    rows_all = np.asarray(A_indices[0], dtype=np.int64)
    cols_all = np.asarray(A_indices[1], dtype=np.int64)
    vals_all = np.asarray(A_values, dtype=np.float32)

    # Per-rank sparse slices + row degrees.
    rank_data = []
    for r in range(NCORES):
        m = (rows_all // CLOC) == r
        rl = (rows_all[m] - r * CLOC).astype(np.int64)
        cc = cols_all[m]
        vv = vals_all[m]
        deg = np.bincount(rl, minlength=CLOC)
        rank_data.append((rl, cc, vv, deg))

    # ---- Round 1: assign each local class row to half A or half B,
    # balancing total degree (each half holds exactly CLOC/2 rows).
    half_of = []          # per core: row -> 0/1
    for r in range(NCORES):
        deg = rank_data[r][3]
        order = np.argsort(-deg, kind="stable")
        loads = np.zeros(2, dtype=np.int64)
        cnts = np.zeros(2, dtype=np.int64)
        hh = np.empty(CLOC, dtype=np.int64)
        for row in order:
            h = int(np.argmin(np.where(cnts < CLOC // 2, loads, np.iinfo(np.int64).max)))
            hh[row] = h
            loads[h] += deg[row]
            cnts[h] += 1
        half_of.append(hh)

    # Source-half of every contribution is now fixed (depends only on the
    # owner core's half assignment).  Compute per-row (degA, degB).
    degAB = []
    for r in range(NCORES):
        rl, cc, vv, _deg = rank_data[r]
        src_half = np.empty(len(cc), dtype=np.int64)
        for r2 in range(NCORES):
            m2 = (cc // CLOC) == r2
            src_half[m2] = half_of[r2][cc[m2] % CLOC]
        dA = np.bincount(rl[src_half == 0], minlength=CLOC)
        dB = np.bincount(rl[src_half == 1], minlength=CLOC)
        degAB.append((dA, dB, src_half))

    # ---- Round 2: within each half, pack rows into 32 blocks of 128 rows,
    # respecting a fixed per-block (capA, capB) chunk-capacity profile so the
    # SPMD chunk structure is identical on every core.  Fat blocks at the end
    # of each half absorb the tails.
    nfat = 4
    while True:
        capA = np.full(NBH, 2 * P, dtype=np.int64)
        capB = np.full(NBH, 2 * P, dtype=np.int64)
        capA[NBH - nfat:] = 3 * P
        capB[NBH - nfat:] = 3 * P
        perms = []
        ok = True
        for r in range(NCORES):
            dA, dB, _ = degAB[r]
            hh = half_of[r]
            perm = np.empty(CLOC, dtype=np.int64)
            for h in range(2):
                rows_h = np.where(hh == h)[0]
                order = np.argsort(-(dA[rows_h] + dB[rows_h]), kind="stable")
                loadsA = np.zeros(NBH, dtype=np.int64)
                loadsB = np.zeros(NBH, dtype=np.int64)
                cnts = np.zeros(NBH, dtype=np.int64)
                for row in rows_h[order]:
                    a, bb = dA[row], dB[row]
                    score = np.maximum((loadsA + a) / capA, (loadsB + bb) / capB)
                    score[cnts >= P] = np.inf
                    score[loadsA + a > capA] = np.inf
                    score[loadsB + bb > capB] = np.inf
                    blk = int(np.argmin(score))
                    if not np.isfinite(score[blk]):
                        ok = False
                        break
                    perm[row] = (h * NBH + blk) * P + cnts[blk]
                    loadsA[blk] += a
                    loadsB[blk] += bb
                    cnts[blk] += 1
                if not ok:
                    break
            if not ok:
                break
            perms.append(perm)
        if ok:
            break
        nfat += 4
        if nfat > NBH:
            raise RuntimeError("packing failed")
    chunks_a = tuple(int(capA[rb % NBH] // P) for rb in range(NB))
    chunks_b = tuple(int(capB[rb % NBH] // P) for rb in range(NB))
    maxch = max(a + b for a, b in zip(chunks_a, chunks_b))
    new2old = [np.argsort(p) for p in perms]

    cumt = np.array(CUMT, dtype=np.int64)
    tch = np.array(TCH, dtype=np.int64)
    ppfx = np.array(PASS_PREFIX[:-1], dtype=np.int64)

    # First sweep: per-core contribution arrays + per-chunk passes.
    core_arr = []
    chunk_pass = np.zeros((NCORES, NB, maxch), dtype=np.int64)
    for r in range(NCORES):
        rl, cc, vv, _deg = rank_data[r]
        rl_new = perms[r][rl]
        rr = cc // CLOC
        lnew = np.empty(len(cc), dtype=np.int64)
        for r2 in range(NCORES):
            m2 = rr == r2
            lnew[m2] = perms[r2][cc[m2] % CLOC]
        stile = lnew // P
        spass = (stile[:, None] >= ppfx[None, :]).sum(axis=1)
        k_src = np.searchsorted(cumt[1:], stile, side="right")
        ag_row = (NCORES * P * cumt[k_src]
                  + rr * P * tch[k_src]
                  + (lnew - P * cumt[k_src]))
        # index relative to the pass's gather window
        win_row = ag_row - np.where(spass >= NPASS_A, HALF_ROWS, 0)
        assert win_row.min() >= 0 and win_row.max() < HALF_ROWS

        blk = rl_new // P
        order2 = np.lexsort((spass, blk))
        blk = blk[order2]
        spass_s = spass[order2]
        rl_new_s = rl_new[order2]
        vv_s = vv[order2]
        win_row_s = win_row[order2]

        # Slot positions: window-A contributions fill the first chunks_a[blk]
        # chunks; window-B contributions start at the B region.
        is_b = (spass_s >= NPASS_A).astype(np.int64)
        ca = np.array(chunks_a)[blk]
        key = blk * 2 + is_b
        counts = np.bincount(key, minlength=2 * NB)
        starts = np.zeros(2 * NB, dtype=np.int64)
        starts[1:] = np.cumsum(counts)[:-1]
        pos_in_grp = np.arange(len(blk)) - starts[key]
        pos = np.where(is_b == 0, pos_in_grp, ca * P + pos_in_grp)
        ch_idx = pos // P
        p_idx = pos % P
        np.maximum.at(chunk_pass[r], (blk, ch_idx), spass_s)
        core_arr.append((blk, ch_idx, p_idx, rl_new_s, vv_s, win_row_s))

    # Merge pass assignment across cores (program structure must be SPMD).
    merged = chunk_pass.max(axis=0)
    passes = []
    for rb in range(NB):
        ps = []
        for ci in range(chunks_a[rb] + chunks_b[rb]):
            if ci < chunks_a[rb]:
                ps.append(min(NPASS_A - 1, int(merged[rb, ci])))
            else:
                ps.append(max(NPASS_A, int(merged[rb, ci])))
        passes.append(tuple(ps))
    passes = tuple(passes)
    profile = (chunks_a, chunks_b, passes)

    chunks = tuple(a + b for a, b in zip(chunks_a, chunks_b))
    order3 = sorted(
        (passes[rb][ci], rb, ci) for rb in range(NB) for ci in range(chunks[rb])
    )
    goff = np.zeros((NB, maxch), dtype=np.int64)
    for gi, (_, rb, ci) in enumerate(order3):
        goff[rb, ci] = gi
    tot_ch = len(order3)

    per_rank = []
    for r in range(NCORES):
        blk, ch_idx, p_idx, rl_new_s, vv_s, win_row_s = core_arr[r]
        gcol = goff[blk, ch_idx]
        sel_host = np.zeros((P, tot_ch * P), dtype=EX_NP)
        sel_host[p_idx, gcol * P + (rl_new_s % P)] = vv_s.astype(EX_NP)
        idx_flat = np.zeros(tot_ch * P, dtype=np.int16)
        idx_flat[gcol * P + p_idx] = win_row_s.astype(np.int16)
        gidxs_host = np.ascontiguousarray(_wrap16(idx_flat))

        rows = slice(r * CLOC, (r + 1) * CLOC)
        wr = wt2_w[rows][new2old[r]]
        wt_host = np.ascontiguousarray(
            wr.reshape(NB, P, ND, P).transpose(0, 3, 2, 1)
        ).reshape(NB, P, D).astype(MM_NP)
        bias_host = np.ascontiguousarray(
            wt2_b[rows][new2old[r]].reshape(NB, P).T)
        per_rank.append({
            "enc": enc_flat,
            "gidx": gidx_host,
            "wt": wt_host,
            "biasv": bias_host,
            "sel": sel_host,
            "gidxs": gidxs_host,
        })
    return per_rank, profile, new2old


def kernel(**inputs) -> np.ndarray:
    per_rank, profile, new2old = _prep_host(
        inputs["enc_out"], inputs["wt2_w"], inputs["wt2_b"],
        inputs["A_values"], inputs["batch_idx"], inputs["tgt"],
        inputs["A_indices"])
    nc = _build_program(profile)
    res = None
    last_exc = None
    for _attempt in range(3):
        try:
            res = run_bass_kernel_spmd(
                nc, per_rank, core_ids=list(range(NCORES)), trace=TRACE)
            break
        except Exception as e:  # transient runtime/collective hiccups
            last_exc = e
    if res is None:
        raise last_exc
    global LAST_RESULTS
    LAST_RESULTS = res
    outT_full = np.empty((C, N), dtype=np.float32)
    for r in range(NCORES):
        outT_full[r * CLOC + new2old[r]] = res.results[r]["outT"]
    return np.ascontiguousarray(outT_full.T)


# revision 21
# speedup vs baseline: 1.0282x; 1.0282x over previous
"""Trainium2 Bass kernel for nn_Enet_81037442941606 (gnn_message_passing).

Computation (reference):
    g   = enc_out[batch_idx, tgt]                      # [N, D] gather
    h0  = batchnorm(g)  (training stats, biased var)   # [N, D]
    h1  = swish(h0 @ wt2_w.T + wt2_b)                  # [N, C]
    out = h1 @ A.T + h1   (A sparse, NNZ entries)      # [N, C]

Strategy (8 NeuronCores, tensor parallel over the class axis):
  * Each core owns a contiguous block of C/8 = 8192 classes: its wt2_w rows,
    its A rows (spmm output rows), and its output columns.
  * The h1^T shard exchange (bf16 AllGather, ~260us of CC ring time) is the
    critical resource.  It is split into 5 chunked collectives over class
    sub-ranges, each fired as soon as the main matmul finishes that
    sub-range, so the CC stream overlaps the matmul and most of the spmm.
  * The spmm runs in 4 passes keyed to AllGather-chunk completion: the host
    sorts each output row-block's contributions by source availability and
    packs them into 128-slot selection chunks; a chunk runs in the earliest
    pass whose ag_out prefix covers all its sources.  Partial psum results
    between passes are parked in SBUF (bf16) and merged back via identity
    matmuls.  Row gathers use the batched dma_gather instruction (int16
    indices, so sources address one of two 32768-row windows of ag_out;
    chunks are kept window-pure by the packing).
  * Host concatenates the 8 output shards and transposes back to [N, C].
"""

import numpy as np
import ml_dtypes

import concourse.bacc as bacc
import concourse.bass as bass
import concourse.mybir as mybir
import concourse.tile as tile
from concourse.bass_utils import run_bass_kernel_spmd
from concourse.masks import make_identity

# Problem sizes (hardcoded per contest rules).
B, S, D, C, N = 32, 128, 1024, 65536, 512
NNZ = 262144
EPS = 1e-5
NCORES = 8
CLOC = C // NCORES          # classes per core = 8192
NB = CLOC // 128            # 64 c-tiles (row-blocks) per core
NBH = NB // 2               # 32 c-tiles per half
ND = D // 128               # 8 contraction chunks
NT = N // 128               # 4 token tiles
P = 128

EX_DT = mybir.dt.bfloat16   # h1 exchange dtype
EX_NP = ml_dtypes.bfloat16
MM_DT = mybir.dt.bfloat16   # main-matmul operand dtype (W, h0^T)
MM_NP = ml_dtypes.bfloat16

# AllGather chunking (c-tiles per chunk) and spmm pass prefixes (c-tiles).
TCH = (8, 24, 24, 8)
CUMT = (0, 8, 32, 56, 64)
NAG = len(TCH)
PASS_PREFIX = (8, 32, 56, 64)       # pass j sources lie in tiles < this
NPASS = len(PASS_PREFIX)
NPASS_A = 2                         # passes sourcing window A (prefix <= 32)
HALF_ROWS = NBH * NCORES * P        # 32768: dma_gather int16 window size
WB = 4                      # W tiles per load group
GB = 4                      # sel-chunks per batched spmm gather
SB = 32                     # sel-chunks per sel-matrix DMA

_PROGRAM_CACHE = {}
TRACE = False          # set by test.py to capture an NTFF profile
LAST_RESULTS = None    # BassKernelResults of the last kernel() call


def _build_program(profile):
    """Build + compile the SPMD Bass program (identical on all 8 cores).

    profile = (chunks_a, chunks_b, passes):
      chunks_a[rb]/chunks_b[rb] = sel chunks sourcing window A (pass 0) /
      window B (passes 1-3) for row block rb; passes[rb] = per-chunk pass.
    """
    if profile in _PROGRAM_CACHE:
        return _PROGRAM_CACHE[profile]
    chunks_a, chunks_b, passes = profile
    chunks = tuple(a + b for a, b in zip(chunks_a, chunks_b))
    tot_ch = sum(chunks)

    # Global chunk order: (pass, rb, chunk_idx).
    order = sorted(
        (passes[rb][ci], rb, ci) for rb in range(NB) for ci in range(chunks[rb])
    )
    gidx_of = {(rb, ci): gi for gi, (_, rb, ci) in enumerate(order)}
    sessions = [[] for _ in range(NPASS)]
    first_pass = {}
    last_pass = {}
    for rb in range(NB):
        ps = passes[rb]
        first_pass[rb] = ps[0]
        last_pass[rb] = ps[-1]
        for j in sorted(set(ps)):
            cis = [ci for ci in range(chunks[rb]) if ps[ci] == j]
            gi0 = gidx_of[(rb, cis[0])]
            sessions[j].append((rb, gi0, gi0 + len(cis)))
    for j in range(NPASS):
        sessions[j].sort(key=lambda s: s[1])
    pass_start = [min((s[1] for s in sessions[j]), default=0)
                  for j in range(NPASS)]

    nc = bacc.Bacc("TRN2", target_bir_lowering=False, debug=False,
                   num_devices=NCORES)
    f32 = mybir.dt.float32
    i16 = mybir.dt.int16

    enc = nc.dram_tensor("enc", [B * S, D], f32, kind="ExternalInput")
    gidx = nc.dram_tensor("gidx", [P, NT * P // 16], i16, kind="ExternalInput")
    wt = nc.dram_tensor("wt", [NB, P, D], MM_DT, kind="ExternalInput")
    biasv = nc.dram_tensor("biasv", [P, NB], f32, kind="ExternalInput")
    sel = nc.dram_tensor("sel", [P, tot_ch * P], EX_DT, kind="ExternalInput")
    gidxs = nc.dram_tensor("gidxs", [P, tot_ch * 8], i16, kind="ExternalInput")
    outT = nc.dram_tensor("outT", [CLOC, N], EX_DT, kind="ExternalOutput")

    ag_ins = [nc.dram_tensor(f"ag_in{k}", [TCH[k] * P, N], EX_DT)
              for k in range(NAG)]
    ag_out = nc.dram_tensor("ag_out", [C, N], EX_DT, addr_space="Shared")
    # ag_out row space is chunk-major: chunk k rows live at
    # 8*P*CUMT[k] + rr*P*TCH[k] + (l - P*CUMT[k]).  Host maps gather
    # indices to this layout, relative to the pass's window base.
    ag_in_vs = [t.ap().rearrange("(i p) n -> p i n", p=P) for t in ag_ins]
    outT_v = outT.ap().rearrange("(i p) n -> i p n", p=P)

    with tile.TileContext(nc) as tc:
        with (
            tc.tile_pool(name="persist", bufs=1) as persist,
        ):
            h0T = persist.tile([P, ND * N], MM_DT)      # [d%128, (dchunk, n)]
            h1T = persist.tile([P, NB * N], EX_DT)      # [c%128, (ctile, n)]
            partial = persist.tile([P, NB * N], EX_DT)  # spmm pass partials
            bias_t = persist.tile([P, NB], f32)
            gidxs_t = persist.tile([P, tot_ch * 8], i16)
            ident = persist.tile([P, P], f32)
            identb = persist.tile([P, P], EX_DT)
            mean_s = persist.tile([P, ND], f32)
            rstd_s = persist.tile([P, ND], f32)
            gidx_t = persist.tile([P, NT * P // 16], i16)

            nc.sync.dma_start(out=gidx_t[:], in_=gidx[:])
            make_identity(nc, ident[:])
            nc.vector.tensor_copy(out=identb[:], in_=ident[:])
            nc.sync.dma_start(out=bias_t[:], in_=biasv[:])
            nc.sync.dma_start(out=gidxs_t[:], in_=gidxs[:])

            # ---------------- Phase A: gather + batchnorm + h0^T -----------
            with (
                tc.tile_pool(name="phA", bufs=1) as phA,
                tc.tile_pool(name="psA", bufs=4, space="PSUM") as psA,
            ):
                g_all = phA.tile([P, NT * D], f32, tag="g")
                nc.gpsimd.dma_gather(
                    out_ap=g_all[:].rearrange("p (b e) -> p b e", e=D),
                    in_ap=enc[:],
                    idxs_ap=gidx_t[:],
                    num_idxs=NT * P,
                    num_idxs_reg=NT * P,
                    elem_size=D,
                )

                # Raw transpose g -> h0T (tokens on the free axis), d-chunk
                # major so per-chunk batch stats chase the transposes.
                sum_s = phA.tile([P, ND], f32, tag="sums")
                sq_s = phA.tile([P, ND], f32, tag="sqs")
                scr = phA.tile([P, N], f32, tag="scr")
                for i in range(ND):
                    for j in range(NT):
                        tp = psA.tile([P, P], f32, space="PSUM", tag="tp")
                        nc.tensor.transpose(
                            tp[:], g_all[:, j * D + i * P: j * D + (i + 1) * P],
                            ident[:])
                        nc.vector.tensor_copy(
                            out=h0T[:, i * N + j * P: i * N + (j + 1) * P],
                            in_=tp[:])
                    nc.scalar.activation(
                        scr[:], h0T[:, i * N:(i + 1) * N],
                        mybir.ActivationFunctionType.Copy,
                        accum_out=sum_s[:, i:i + 1])
                    nc.scalar.activation(
                        scr[:], h0T[:, i * N:(i + 1) * N],
                        mybir.ActivationFunctionType.Square,
                        accum_out=sq_s[:, i:i + 1])

                ex2_s = phA.tile([P, ND], f32, tag="ex2")
                var_s = phA.tile([P, ND], f32, tag="var")
                nc.scalar.mul(mean_s[:], sum_s[:], 1.0 / N)
                nc.scalar.mul(ex2_s[:], sq_s[:], 1.0 / N)
                nc.vector.tensor_tensor(
                    out=var_s[:], in0=mean_s[:], in1=mean_s[:],
                    op=mybir.AluOpType.mult)
                nc.vector.tensor_tensor(
                    out=var_s[:], in0=ex2_s[:], in1=var_s[:],
                    op=mybir.AluOpType.subtract)
                sd_s = phA.tile([P, ND], f32, tag="sd")
                epsb = phA.tile([P, 1], f32, tag="epsb")
                nc.vector.memset(epsb[:], EPS)
                nc.scalar.activation(
                    sd_s[:], var_s[:], mybir.ActivationFunctionType.Sqrt,
                    bias=epsb[:, :1], scale=1.0)
                nc.vector.reciprocal(rstd_s[:], sd_s[:])

                for i in range(ND):
                    nc.vector.tensor_scalar(
                        out=h0T[:, i * N:(i + 1) * N],
                        in0=h0T[:, i * N:(i + 1) * N],
                        scalar1=mean_s[:, i:i + 1],
                        scalar2=rstd_s[:, i:i + 1],
                        op0=mybir.AluOpType.subtract,
                        op1=mybir.AluOpType.mult,
                    )

            # ---------------- Phases B + D interleaved ---------------------
            # B: h1^T = swish(W h0^T + b), in AG-chunk order; each chunked
            # AllGather fires as soon as its c-tile range is in ag_in[k].
            # D: spmm passes are emitted between B chunks so the gpsimd
            # engine can start pass-j gathers as soon as AG chunk j lands,
            # without blocking later AG triggers (in-order engines).
            # Finals (which read h1T residuals) only occur in passes >= 2,
            # after all of B has been emitted.
            assert all(ps[-1] >= NPASS_A for ps in passes)
            wt_b = wt.ap().rearrange("(a b) p d -> a b p d", b=WB)
            with (
                tc.tile_pool(name="phB", bufs=2) as phB,
                tc.tile_pool(name="psB", bufs=4, space="PSUM") as psB,
                tc.tile_pool(name="ctp", bufs=7) as ctp,
                tc.tile_pool(name="selp", bufs=2) as selp,
                tc.tile_pool(name="otp", bufs=3) as otp,
                tc.tile_pool(name="psD", bufs=4, space="PSUM") as psD,
            ):
                ct_tiles = {}
                sel_tiles = {}

                def emit_b_chunk(k):
                    for a in range(CUMT[k] // WB, CUMT[k + 1] // WB):
                        wt_a = phB.tile([P, WB * D], MM_DT, tag="wt")
                        nc.sync.dma_start(
                            out=wt_a[:].rearrange("p (b d) -> p b d", b=WB),
                            in_=wt_b[a].rearrange("b p d -> p b d"))
                        for bsub in range(WB):
                            i = a * WB + bsub
                            h1ps = psB.tile([P, N], f32, space="PSUM",
                                            tag="h1ps")
                            for kk in range(ND):
                                nc.tensor.matmul(
                                    out=h1ps[:],
                                    lhsT=wt_a[:, bsub * D + kk * P:
                                              bsub * D + (kk + 1) * P],
                                    rhs=h0T[:, kk * N:(kk + 1) * N],
                                    start=(kk == 0), stop=(kk == ND - 1),
                                )
                            nc.scalar.activation(
                                h1T[:, i * N:(i + 1) * N], h1ps[:],
                                mybir.ActivationFunctionType.Silu,
                                bias=bias_t[:, i:i + 1], scale=1.0)
                        g0 = a * WB
                        nc.sync.dma_start(
                            out=ag_in_vs[k][:, g0 - CUMT[k]:
                                            g0 - CUMT[k] + WB, :],
                            in_=h1T[:, g0 * N:(g0 + WB) * N].rearrange(
                                "p (i n) -> p i n", n=N))
                    r0, r1 = NCORES * P * CUMT[k], NCORES * P * CUMT[k + 1]
                    nc.gpsimd.collective_compute(
                        "AllGather",
                        mybir.AluOpType.bypass,
                        replica_groups=[list(range(NCORES))],
                        ins=[ag_ins[k][:].opt()],
                        outs=[ag_out[r0:r1, :].opt()],
                    )

                def ensure_ct(j, b, pend):
                    key = (j, b)
                    if key in ct_tiles:
                        return ct_tiles[key]
                    a0 = pass_start[j] + b * GB
                    a1 = min(a0 + GB, pend)
                    m = a1 - a0
                    t = ctp.tile([P, GB * N], EX_DT, tag="ct")
                    base = 0 if j < NPASS_A else HALF_ROWS
                    ext = NCORES * P * PASS_PREFIX[j] - base
                    nc.gpsimd.dma_gather(
                        out_ap=t[:, :m * N].rearrange("p (b n) -> p b n", n=N),
                        in_ap=ag_out[base:base + ext, :],
                        idxs_ap=gidxs_t[:, a0 * 8:a1 * 8],
                        num_idxs=m * P,
                        num_idxs_reg=m * P,
                        elem_size=N,
                    )
                    ct_tiles[key] = t
                    return t

                def ensure_sel(b):
                    if b in sel_tiles:
                        return sel_tiles[b]
                    a0 = b * SB
                    a1 = min(a0 + SB, tot_ch)
                    m = a1 - a0
                    t = selp.tile([P, SB * P], EX_DT, tag="sel")
                    nc.sync.dma_start(
                        out=t[:, :m * P], in_=sel[:, a0 * P:a1 * P])
                    sel_tiles[b] = t
                    return t

                def emit_d_pass(j):
                    if not sessions[j]:
                        return
                    pend = max(s[2] for s in sessions[j])
                    for rb, gi0, gi1 in sessions[j]:
                        has_prev = j > first_pass[rb]
                        is_last = j == last_pass[rb]
                        acc = psD.tile([P, N], f32, space="PSUM", tag="acc")
                        for gi in range(gi0, gi1):
                            ctb = (gi - pass_start[j]) // GB
                            cto = gi - pass_start[j] - ctb * GB
                            ct = ensure_ct(j, ctb, pend)
                            slb, slo = gi // SB, gi % SB
                            sl = ensure_sel(slb)
                            last_mm = (gi == gi1 - 1) and not has_prev
                            nc.tensor.matmul(
                                out=acc[:],
                                lhsT=sl[:, slo * P:(slo + 1) * P],
                                rhs=ct[:, cto * N:(cto + 1) * N],
                                start=(gi == gi0), stop=last_mm,
                            )
                        if has_prev:
                            nc.tensor.matmul(
                                out=acc[:],
                                lhsT=identb[:],
                                rhs=partial[:, rb * N:(rb + 1) * N],
                                start=False, stop=True,
                            )
                        if is_last:
                            o_t = otp.tile([P, N], EX_DT, tag="ot")
                            nc.vector.tensor_tensor(
                                out=o_t[:], in0=acc[:],
                                in1=h1T[:, rb * N:(rb + 1) * N],
                                op=mybir.AluOpType.add)
                            nc.sync.dma_start(out=outT_v[rb], in_=o_t[:])
                        else:
                            nc.scalar.activation(
                                partial[:, rb * N:(rb + 1) * N], acc[:],
                                mybir.ActivationFunctionType.Copy)

                for k in range(NAG):
                    emit_b_chunk(k)
                for j in range(NPASS):
                    emit_d_pass(j)

    nc.compile()
    _PROGRAM_CACHE[profile] = nc
    return nc


def _wrap16(idx_flat):
    """Layout flat gather indices for dma_gather: idx i -> [i%16, i//16],
    replicated across the 8 groups of 16 partitions."""
    n = len(idx_flat)
    assert n % 16 == 0
    a = np.asarray(idx_flat, dtype=np.int16).reshape(n // 16, 16).T  # [16, n/16]
    return np.tile(a, (8, 1))                                        # [128, n/16]


def _prep_host(enc_out, wt2_w, wt2_b, A_values, batch_idx, tgt, A_indices):
    """Shard inputs + restructure the sparse matrix for the device program."""
    enc_flat = np.ascontiguousarray(
        np.asarray(enc_out, dtype=np.float32).reshape(B * S, D))
    flat_idx = (np.asarray(batch_idx, dtype=np.int64) * S
                + np.asarray(tgt, dtype=np.int64))
    gidx_host = np.ascontiguousarray(_wrap16(flat_idx))

    wt2_w = np.asarray(wt2_w, dtype=np.float32)
    wt2_b = np.asarray(wt2_b, dtype=np.float32)
    rows_all = np.asarray(A_indices[0], dtype=np.int64)
    cols_all = np.asarray(A_indices[1], dtype=np.int64)
    vals_all = np.asarray(A_values, dtype=np.float32)

    # Per-rank sparse slices + row degrees.
    rank_data = []
    for r in range(NCORES):
        m = (rows_all // CLOC) == r
        rl = (rows_all[m] - r * CLOC).astype(np.int64)
        cc = cols_all[m]
        vv = vals_all[m]
        deg = np.bincount(rl, minlength=CLOC)
        rank_data.append((rl, cc, vv, deg))

    # ---- Round 1: assign each local class row to half A or half B,
    # balancing total degree (each half holds exactly CLOC/2 rows).
    half_of = []          # per core: row -> 0/1
    for r in range(NCORES):
        deg = rank_data[r][3]
        order = np.argsort(-deg, kind="stable")
        loads = np.zeros(2, dtype=np.int64)
        cnts = np.zeros(2, dtype=np.int64)
        hh = np.empty(CLOC, dtype=np.int64)
        for row in order:
            h = int(np.argmin(np.where(cnts < CLOC // 2, loads, np.iinfo(np.int64).max)))
            hh[row] = h
            loads[h] += deg[row]
            cnts[h] += 1
        half_of.append(hh)

    # Source-half of every contribution is now fixed (depends only on the
    # owner core's half assignment).  Compute per-row (degA, degB).
    degAB = []
    for r in range(NCORES):
        rl, cc, vv, _deg = rank_data[r]
        src_half = np.empty(len(cc), dtype=np.int64)
        for r2 in range(NCORES):
            m2 = (cc // CLOC) == r2
            src_half[m2] = half_of[r2][cc[m2] % CLOC]
        dA = np.bincount(rl[src_half == 0], minlength=CLOC)
        dB = np.bincount(rl[src_half == 1], minlength=CLOC)
        degAB.append((dA, dB, src_half))

    # ---- Round 2: within each half, pack rows into 32 blocks of 128 rows,
    # respecting a fixed per-block (capA, capB) chunk-capacity profile so the
    # SPMD chunk structure is identical on every core.  Fat blocks at the end
    # of each half absorb the tails.
    nfat = 4
    while True:
        capA = np.full(NBH, 2 * P, dtype=np.int64)
        capB = np.full(NBH, 2 * P, dtype=np.int64)
        capA[NBH - nfat:] = 3 * P
        capB[NBH - nfat:] = 3 * P
        perms = []
        ok = True
        for r in range(NCORES):
            dA, dB, _ = degAB[r]
            hh = half_of[r]
            perm = np.empty(CLOC, dtype=np.int64)
            for h in range(2):
                rows_h = np.where(hh == h)[0]
                order = np.argsort(-(dA[rows_h] + dB[rows_h]), kind="stable")
                loadsA = np.zeros(NBH, dtype=np.int64)
                loadsB = np.zeros(NBH, dtype=np.int64)
                cnts = np.zeros(NBH, dtype=np.int64)
                for row in rows_h[order]:
                    a, bb = dA[row], dB[row]
                    score = np.maximum((loadsA + a) / capA, (loadsB + bb) / capB)
                    score[cnts >= P] = np.inf
                    score[loadsA + a > capA] = np.inf
                    score[loadsB + bb > capB] = np.inf
                    blk = int(np.argmin(score))
                    if not np.isfinite(score[blk]):
                        ok = False
                        break
                    perm[row] = (h * NBH + blk) * P + cnts[blk]
                    loadsA[blk] += a
                    loadsB[blk] += bb
                    cnts[blk] += 1
                if not ok:
                    break
            if not ok:
                break
            perms.append(perm)
        if ok:
            break
        nfat += 4
        if nfat > NBH:
            raise RuntimeError("packing failed")
    chunks_a = tuple(int(capA[rb % NBH] // P) for rb in range(NB))
    chunks_b = tuple(int(capB[rb % NBH] // P) for rb in range(NB))
    maxch = max(a + b for a, b in zip(chunks_a, chunks_b))
    new2old = [np.argsort(p) for p in perms]

    cumt = np.array(CUMT, dtype=np.int64)
    tch = np.array(TCH, dtype=np.int64)
    ppfx = np.array(PASS_PREFIX[:-1], dtype=np.int64)

    # First sweep: per-core contribution arrays + per-chunk passes.
    core_arr = []
    chunk_pass = np.zeros((NCORES, NB, maxch), dtype=np.int64)
    for r in range(NCORES):
        rl, cc, vv, _deg = rank_data[r]
        rl_new = perms[r][rl]
        rr = cc // CLOC
        lnew = np.empty(len(cc), dtype=np.int64)
        for r2 in range(NCORES):
            m2 = rr == r2
            lnew[m2] = perms[r2][cc[m2] % CLOC]
        stile = lnew // P
        spass = (stile[:, None] >= ppfx[None, :]).sum(axis=1)
        k_src = np.searchsorted(cumt[1:], stile, side="right")
        ag_row = (NCORES * P * cumt[k_src]
                  + rr * P * tch[k_src]
                  + (lnew - P * cumt[k_src]))
        # index relative to the pass's gather window
        win_row = ag_row - np.where(spass >= NPASS_A, HALF_ROWS, 0)
        assert win_row.min() >= 0 and win_row.max() < HALF_ROWS

        blk = rl_new // P
        order2 = np.lexsort((spass, blk))
        blk = blk[order2]
        spass_s = spass[order2]
        rl_new_s = rl_new[order2]
        vv_s = vv[order2]
        win_row_s = win_row[order2]

        # Slot positions: window-A contributions fill the first chunks_a[blk]
        # chunks; window-B contributions start at the B region.
        is_b = (spass_s >= NPASS_A).astype(np.int64)
        ca = np.array(chunks_a)[blk]
        key = blk * 2 + is_b
        counts = np.bincount(key, minlength=2 * NB)
        starts = np.zeros(2 * NB, dtype=np.int64)
        starts[1:] = np.cumsum(counts)[:-1]
        pos_in_grp = np.arange(len(blk)) - starts[key]
        pos = np.where(is_b == 0, pos_in_grp, ca * P + pos_in_grp)
        ch_idx = pos // P
        p_idx = pos % P
        np.maximum.at(chunk_pass[r], (blk, ch_idx), spass_s)
        core_arr.append((blk, ch_idx, p_idx, rl_new_s, vv_s, win_row_s))

    # Merge pass assignment across cores (program structure must be SPMD).
    merged = chunk_pass.max(axis=0)
    passes = []
    for rb in range(NB):
        ps = []
        for ci in range(chunks_a[rb] + chunks_b[rb]):
            if ci < chunks_a[rb]:
                ps.append(min(NPASS_A - 1, int(merged[rb, ci])))
            else:
                ps.append(max(NPASS_A, int(merged[rb, ci])))
        passes.append(tuple(ps))
    passes = tuple(passes)
    profile = (chunks_a, chunks_b, passes)

    chunks = tuple(a + b for a, b in zip(chunks_a, chunks_b))
    order3 = sorted(
        (passes[rb][ci], rb, ci) for rb in range(NB) for ci in range(chunks[rb])
    )
    goff = np.zeros((NB, maxch), dtype=np.int64)
    for gi, (_, rb, ci) in enumerate(order3):
        goff[rb, ci] = gi
    tot_ch = len(order3)

    per_rank = []
    for r in range(NCORES):
        blk, ch_idx, p_idx, rl_new_s, vv_s, win_row_s = core_arr[r]
        gcol = goff[blk, ch_idx]
        sel_host = np.zeros((P, tot_ch * P), dtype=EX_NP)
        sel_host[p_idx, gcol * P + (rl_new_s % P)] = vv_s.astype(EX_NP)
        idx_flat = np.zeros(tot_ch * P, dtype=np.int16)
        idx_flat[gcol * P + p_idx] = win_row_s.astype(np.int16)
        gidxs_host = np.ascontiguousarray(_wrap16(idx_flat))

        rows = slice(r * CLOC, (r + 1) * CLOC)
        wr = wt2_w[rows][new2old[r]]
        wt_host = np.ascontiguousarray(
            wr.reshape(NB, P, ND, P).transpose(0, 3, 2, 1)
        ).reshape(NB, P, D).astype(MM_NP)
        bias_host = np.ascontiguousarray(
            wt2_b[rows][new2old[r]].reshape(NB, P).T)
        per_rank.append({
            "enc": enc_flat,
            "gidx": gidx_host,
            "wt": wt_host,
            "biasv": bias_host,
            "sel": sel_host,
            "gidxs": gidxs_host,
        })
    return per_rank, profile, new2old


def kernel(**inputs) -> np.ndarray:
    per_rank, profile, new2old = _prep_host(
        inputs["enc_out"], inputs["wt2_w"], inputs["wt2_b"],
        inputs["A_values"], inputs["batch_idx"], inputs["tgt"],
        inputs["A_indices"])
    nc = _build_program(profile)
    res = None
    last_exc = None
    for _attempt in range(3):
        try:
            res = run_bass_kernel_spmd(
                nc, per_rank, core_ids=list(range(NCORES)), trace=TRACE)
            break
        except Exception as e:  # transient runtime/collective hiccups
            last_exc = e
    if res is None:
        raise last_exc
    global LAST_RESULTS
    LAST_RESULTS = res
    outT_full = np.empty((C, N), dtype=np.float32)
    for r in range(NCORES):
        outT_full[r * CLOC + new2old[r]] = res.results[r]["outT"].astype(np.float32)
    return np.ascontiguousarray(outT_full.T)


# revision 23
# speedup vs baseline: 1.0622x; 1.0330x over previous
"""Trainium2 Bass kernel for nn_Enet_81037442941606 (gnn_message_passing).

Computation (reference):
    g   = enc_out[batch_idx, tgt]                      # [N, D] gather
    h0  = batchnorm(g)  (training stats, biased var)   # [N, D]
    h1  = swish(h0 @ wt2_w.T + wt2_b)                  # [N, C]
    out = h1 @ A.T + h1   (A sparse, NNZ entries)      # [N, C]

Strategy (8 NeuronCores, tensor parallel over the class axis):
  * Each core owns a contiguous block of C/8 = 8192 classes: its wt2_w rows,
    its A rows (spmm output rows), and its output columns.
  * The h1^T shard exchange (bf16 AllGather, ~260us of CC ring time) is the
    critical resource.  It is split into 5 chunked collectives over class
    sub-ranges, each fired as soon as the main matmul finishes that
    sub-range, so the CC stream overlaps the matmul and most of the spmm.
  * The spmm runs in 4 passes keyed to AllGather-chunk completion: the host
    sorts each output row-block's contributions by source availability and
    packs them into 128-slot selection chunks; a chunk runs in the earliest
    pass whose ag_out prefix covers all its sources.  Partial psum results
    between passes are parked in SBUF (bf16) and merged back via identity
    matmuls.  Row gathers use the batched dma_gather instruction (int16
    indices, so sources address one of two 32768-row windows of ag_out;
    chunks are kept window-pure by the packing).
  * Host concatenates the 8 output shards and transposes back to [N, C].
"""

import numpy as np
import ml_dtypes

import concourse.bacc as bacc
import concourse.bass as bass
import concourse.mybir as mybir
import concourse.tile as tile
from concourse.bass_utils import run_bass_kernel_spmd
from concourse.masks import make_identity

# Problem sizes (hardcoded per contest rules).
B, S, D, C, N = 32, 128, 1024, 65536, 512
NNZ = 262144
EPS = 1e-5
NCORES = 8
CLOC = C // NCORES          # classes per core = 8192
NB = CLOC // 128            # 64 c-tiles (row-blocks) per core
NBH = NB // 2               # 32 c-tiles per half
ND = D // 128               # 8 contraction chunks
NT = N // 128               # 4 token tiles
P = 128

EX_DT = mybir.dt.bfloat16   # h1 exchange dtype
EX_NP = ml_dtypes.bfloat16
MM_DT = mybir.dt.bfloat16   # main-matmul operand dtype (W, h0^T)
MM_NP = ml_dtypes.bfloat16

# AllGather chunking (c-tiles per chunk) and spmm pass prefixes (c-tiles).
TCH = (8, 24, 24, 8)
CUMT = (0, 8, 32, 56, 64)
NAG = len(TCH)
PASS_PREFIX = (8, 32, 56, 64)       # pass j sources lie in tiles < this
NPASS = len(PASS_PREFIX)
NPASS_A = 2                         # passes sourcing window A (prefix <= 32)
HALF_ROWS = NBH * NCORES * P        # 32768: dma_gather int16 window size
WB = 4                      # W tiles per load group
GB = 4                      # sel-chunks per batched spmm gather
SB = 32                     # sel-chunks per sel-matrix DMA

_PROGRAM_CACHE = {}
TRACE = False          # set by test.py to capture an NTFF profile
LAST_RESULTS = None    # BassKernelResults of the last kernel() call


def _build_program(profile):
    """Build + compile the SPMD Bass program (identical on all 8 cores).

    profile = (chunks_a, chunks_b, passes):
      chunks_a[rb]/chunks_b[rb] = sel chunks sourcing window A (pass 0) /
      window B (passes 1-3) for row block rb; passes[rb] = per-chunk pass.
    """
    if profile in _PROGRAM_CACHE:
        return _PROGRAM_CACHE[profile]
    chunks_a, chunks_b, passes = profile
    chunks = tuple(a + b for a, b in zip(chunks_a, chunks_b))
    tot_ch = sum(chunks)

    # Global chunk order: (pass, rb, chunk_idx).
    order = sorted(
        (passes[rb][ci], rb, ci) for rb in range(NB) for ci in range(chunks[rb])
    )
    gidx_of = {(rb, ci): gi for gi, (_, rb, ci) in enumerate(order)}
    sessions = [[] for _ in range(NPASS)]
    first_pass = {}
    last_pass = {}
    for rb in range(NB):
        ps = passes[rb]
        first_pass[rb] = ps[0]
        last_pass[rb] = ps[-1]
        for j in sorted(set(ps)):
            cis = [ci for ci in range(chunks[rb]) if ps[ci] == j]
            gi0 = gidx_of[(rb, cis[0])]
            sessions[j].append((rb, gi0, gi0 + len(cis)))
    for j in range(NPASS):
        sessions[j].sort(key=lambda s: s[1])
    pass_start = [min((s[1] for s in sessions[j]), default=0)
                  for j in range(NPASS)]

    nc = bacc.Bacc("TRN2", target_bir_lowering=False, debug=False,
                   num_devices=NCORES)
    f32 = mybir.dt.float32
    i16 = mybir.dt.int16

    enc = nc.dram_tensor("enc", [B * S, D], f32, kind="ExternalInput")
    gidx = nc.dram_tensor("gidx", [P, NT * P // 16], i16, kind="ExternalInput")
    wt = nc.dram_tensor("wt", [NB, P, D], MM_DT, kind="ExternalInput")
    biasv = nc.dram_tensor("biasv", [P, NB], f32, kind="ExternalInput")
    selv = nc.dram_tensor("selv", [P, tot_ch], f32, kind="ExternalInput")
    seld = nc.dram_tensor("seld", [P, tot_ch], f32, kind="ExternalInput")
    iotav = nc.dram_tensor("iotav", [P, P], f32, kind="ExternalInput")
    gidxs = nc.dram_tensor("gidxs", [P, tot_ch * 8], i16, kind="ExternalInput")
    outT = nc.dram_tensor("outT", [CLOC, N], EX_DT, kind="ExternalOutput")

    ag_ins = [nc.dram_tensor(f"ag_in{k}", [TCH[k] * P, N], EX_DT)
              for k in range(NAG)]
    ag_out = nc.dram_tensor("ag_out", [C, N], EX_DT, addr_space="Shared")
    # ag_out row space is chunk-major: chunk k rows live at
    # 8*P*CUMT[k] + rr*P*TCH[k] + (l - P*CUMT[k]).  Host maps gather
    # indices to this layout, relative to the pass's window base.
    ag_in_vs = [t.ap().rearrange("(i p) n -> p i n", p=P) for t in ag_ins]
    outT_v = outT.ap().rearrange("(i p) n -> i p n", p=P)

    with tile.TileContext(nc) as tc:
        with (
            tc.tile_pool(name="persist", bufs=1) as persist,
        ):
            h0T = persist.tile([P, ND * N], MM_DT)      # [d%128, (dchunk, n)]
            h1T = persist.tile([P, NB * N], EX_DT)      # [c%128, (ctile, n)]
            partial = persist.tile([P, NB * N], EX_DT)  # spmm pass partials
            bias_t = persist.tile([P, NB], f32)
            gidxs_t = persist.tile([P, tot_ch * 8], i16)
            selv_t = persist.tile([P, tot_ch], f32)
            seld_t = persist.tile([P, tot_ch], f32)
            iota_t = persist.tile([P, P], f32)
            ident = persist.tile([P, P], f32)
            identb = persist.tile([P, P], EX_DT)
            mean_s = persist.tile([P, ND], f32)
            rstd_s = persist.tile([P, ND], f32)
            gidx_t = persist.tile([P, NT * P // 16], i16)

            nc.sync.dma_start(out=gidx_t[:], in_=gidx[:])
            make_identity(nc, ident[:])
            nc.vector.tensor_copy(out=identb[:], in_=ident[:])
            nc.sync.dma_start(out=bias_t[:], in_=biasv[:])
            nc.sync.dma_start(out=gidxs_t[:], in_=gidxs[:])
            nc.sync.dma_start(out=selv_t[:], in_=selv[:])
            nc.sync.dma_start(out=seld_t[:], in_=seld[:])
            nc.sync.dma_start(out=iota_t[:], in_=iotav[:])

            # ---------------- Phase A: gather + batchnorm + h0^T -----------
            with (
                tc.tile_pool(name="phA", bufs=1) as phA,
                tc.tile_pool(name="psA", bufs=4, space="PSUM") as psA,
            ):
                g_all = phA.tile([P, NT * D], f32, tag="g")
                nc.gpsimd.dma_gather(
                    out_ap=g_all[:].rearrange("p (b e) -> p b e", e=D),
                    in_ap=enc[:],
                    idxs_ap=gidx_t[:],
                    num_idxs=NT * P,
                    num_idxs_reg=NT * P,
                    elem_size=D,
                )

                # Raw transpose g -> h0T (tokens on the free axis), d-chunk
                # major so per-chunk batch stats chase the transposes.
                sum_s = phA.tile([P, ND], f32, tag="sums")
                sq_s = phA.tile([P, ND], f32, tag="sqs")
                scr = phA.tile([P, N], f32, tag="scr")
                for i in range(ND):
                    for j in range(NT):
                        tp = psA.tile([P, P], f32, space="PSUM", tag="tp")
                        nc.tensor.transpose(
                            tp[:], g_all[:, j * D + i * P: j * D + (i + 1) * P],
                            ident[:])
                        nc.vector.tensor_copy(
                            out=h0T[:, i * N + j * P: i * N + (j + 1) * P],
                            in_=tp[:])
                    nc.scalar.activation(
                        scr[:], h0T[:, i * N:(i + 1) * N],
                        mybir.ActivationFunctionType.Copy,
                        accum_out=sum_s[:, i:i + 1])
                    nc.scalar.activation(
                        scr[:], h0T[:, i * N:(i + 1) * N],
                        mybir.ActivationFunctionType.Square,
                        accum_out=sq_s[:, i:i + 1])

                ex2_s = phA.tile([P, ND], f32, tag="ex2")
                var_s = phA.tile([P, ND], f32, tag="var")
                nc.scalar.mul(mean_s[:], sum_s[:], 1.0 / N)
                nc.scalar.mul(ex2_s[:], sq_s[:], 1.0 / N)
                nc.vector.tensor_tensor(
                    out=var_s[:], in0=mean_s[:], in1=mean_s[:],
                    op=mybir.AluOpType.mult)
                nc.vector.tensor_tensor(
                    out=var_s[:], in0=ex2_s[:], in1=var_s[:],
                    op=mybir.AluOpType.subtract)
                sd_s = phA.tile([P, ND], f32, tag="sd")
                epsb = phA.tile([P, 1], f32, tag="epsb")
                nc.vector.memset(epsb[:], EPS)
                nc.scalar.activation(
                    sd_s[:], var_s[:], mybir.ActivationFunctionType.Sqrt,
                    bias=epsb[:, :1], scale=1.0)
                nc.vector.reciprocal(rstd_s[:], sd_s[:])

                for i in range(ND):
                    nc.vector.tensor_scalar(
                        out=h0T[:, i * N:(i + 1) * N],
                        in0=h0T[:, i * N:(i + 1) * N],
                        scalar1=mean_s[:, i:i + 1],
                        scalar2=rstd_s[:, i:i + 1],
                        op0=mybir.AluOpType.subtract,
                        op1=mybir.AluOpType.mult,
                    )

            # ---------------- Phases B + D interleaved ---------------------
            # B: h1^T = swish(W h0^T + b), in AG-chunk order; each chunked
            # AllGather fires as soon as its c-tile range is in ag_in[k].
            # D: spmm passes are emitted between B chunks so the gpsimd
            # engine can start pass-j gathers as soon as AG chunk j lands,
            # without blocking later AG triggers (in-order engines).
            # Finals (which read h1T residuals) only occur in passes >= 2,
            # after all of B has been emitted.
            assert all(ps[-1] >= NPASS_A for ps in passes)
            wt_b = wt.ap().rearrange("(a b) p d -> a b p d", b=WB)
            with (
                tc.tile_pool(name="phB", bufs=2) as phB,
                tc.tile_pool(name="psB", bufs=4, space="PSUM") as psB,
                tc.tile_pool(name="ctp", bufs=7) as ctp,
                tc.tile_pool(name="selp", bufs=6) as selp,
                tc.tile_pool(name="otp", bufs=3) as otp,
                tc.tile_pool(name="psD", bufs=4, space="PSUM") as psD,
            ):
                ct_tiles = {}
                sel_tiles = {}

                def emit_b_chunk(k):
                    for a in range(CUMT[k] // WB, CUMT[k + 1] // WB):
                        wt_a = phB.tile([P, WB * D], MM_DT, tag="wt")
                        nc.sync.dma_start(
                            out=wt_a[:].rearrange("p (b d) -> p b d", b=WB),
                            in_=wt_b[a].rearrange("b p d -> p b d"))
                        for bsub in range(WB):
                            i = a * WB + bsub
                            h1ps = psB.tile([P, N], f32, space="PSUM",
                                            tag="h1ps")
                            for kk in range(ND):
                                nc.tensor.matmul(
                                    out=h1ps[:],
                                    lhsT=wt_a[:, bsub * D + kk * P:
                                              bsub * D + (kk + 1) * P],
                                    rhs=h0T[:, kk * N:(kk + 1) * N],
                                    start=(kk == 0), stop=(kk == ND - 1),
                                )
                            nc.scalar.activation(
                                h1T[:, i * N:(i + 1) * N], h1ps[:],
                                mybir.ActivationFunctionType.Silu,
                                bias=bias_t[:, i:i + 1], scale=1.0)
                        g0 = a * WB
                        nc.sync.dma_start(
                            out=ag_in_vs[k][:, g0 - CUMT[k]:
                                            g0 - CUMT[k] + WB, :],
                            in_=h1T[:, g0 * N:(g0 + WB) * N].rearrange(
                                "p (i n) -> p i n", n=N))
                    r0, r1 = NCORES * P * CUMT[k], NCORES * P * CUMT[k + 1]
                    nc.gpsimd.collective_compute(
                        "AllGather",
                        mybir.AluOpType.bypass,
                        replica_groups=[list(range(NCORES))],
                        ins=[ag_ins[k][:].opt()],
                        outs=[ag_out[r0:r1, :].opt()],
                    )

                def ensure_ct(j, b, pend):
                    key = (j, b)
                    if key in ct_tiles:
                        return ct_tiles[key]
                    a0 = pass_start[j] + b * GB
                    a1 = min(a0 + GB, pend)
                    m = a1 - a0
                    t = ctp.tile([P, GB * N], EX_DT, tag="ct")
                    base = 0 if j < NPASS_A else HALF_ROWS
                    ext = NCORES * P * PASS_PREFIX[j] - base
                    nc.gpsimd.dma_gather(
                        out_ap=t[:, :m * N].rearrange("p (b n) -> p b n", n=N),
                        in_ap=ag_out[base:base + ext, :],
                        idxs_ap=gidxs_t[:, a0 * 8:a1 * 8],
                        num_idxs=m * P,
                        num_idxs_reg=m * P,
                        elem_size=N,
                    )
                    ct_tiles[key] = t
                    return t

                def make_sel(gi):
                    # Expand chunk gi's selection matrix on-device:
                    # sel[p, c] = (c == dst[p]) * val[p]
                    t = selp.tile([P, P], EX_DT, tag="sel")
                    nc.vector.tensor_scalar(
                        out=t[:], in0=iota_t[:],
                        scalar1=seld_t[:, gi:gi + 1],
                        scalar2=selv_t[:, gi:gi + 1],
                        op0=mybir.AluOpType.is_equal,
                        op1=mybir.AluOpType.mult)
                    return t

                def emit_d_pass(j):
                    if not sessions[j]:
                        return
                    pend = max(s[2] for s in sessions[j])
                    for rb, gi0, gi1 in sessions[j]:
                        has_prev = j > first_pass[rb]
                        is_last = j == last_pass[rb]
                        acc = psD.tile([P, N], f32, space="PSUM", tag="acc")
                        for gi in range(gi0, gi1):
                            ctb = (gi - pass_start[j]) // GB
                            cto = gi - pass_start[j] - ctb * GB
                            ct = ensure_ct(j, ctb, pend)
                            sl = make_sel(gi)
                            last_mm = (gi == gi1 - 1) and not has_prev
                            nc.tensor.matmul(
                                out=acc[:],
                                lhsT=sl[:],
                                rhs=ct[:, cto * N:(cto + 1) * N],
                                start=(gi == gi0), stop=last_mm,
                            )
                        if has_prev:
                            nc.tensor.matmul(
                                out=acc[:],
                                lhsT=identb[:],
                                rhs=partial[:, rb * N:(rb + 1) * N],
                                start=False, stop=True,
                            )
                        if is_last:
                            o_t = otp.tile([P, N], EX_DT, tag="ot")
                            nc.vector.tensor_tensor(
                                out=o_t[:], in0=acc[:],
                                in1=h1T[:, rb * N:(rb + 1) * N],
                                op=mybir.AluOpType.add)
                            nc.sync.dma_start(out=outT_v[rb], in_=o_t[:])
                        else:
                            nc.scalar.activation(
                                partial[:, rb * N:(rb + 1) * N], acc[:],
                                mybir.ActivationFunctionType.Copy)

                for k in range(NAG):
                    emit_b_chunk(k)
                for j in range(NPASS):
                    emit_d_pass(j)

    nc.compile()
    _PROGRAM_CACHE[profile] = nc
    return nc


def _wrap16(idx_flat):
    """Layout flat gather indices for dma_gather: idx i -> [i%16, i//16],
    replicated across the 8 groups of 16 partitions."""
    n = len(idx_flat)
    assert n % 16 == 0
    a = np.asarray(idx_flat, dtype=np.int16).reshape(n // 16, 16).T  # [16, n/16]
    return np.tile(a, (8, 1))                                        # [128, n/16]


def _prep_host(enc_out, wt2_w, wt2_b, A_values, batch_idx, tgt, A_indices):
    """Shard inputs + restructure the sparse matrix for the device program."""
    enc_flat = np.ascontiguousarray(
        np.asarray(enc_out, dtype=np.float32).reshape(B * S, D))
    flat_idx = (np.asarray(batch_idx, dtype=np.int64) * S
                + np.asarray(tgt, dtype=np.int64))
    gidx_host = np.ascontiguousarray(_wrap16(flat_idx))

    wt2_w = np.asarray(wt2_w, dtype=np.float32)
    wt2_b = np.asarray(wt2_b, dtype=np.float32)
    rows_all = np.asarray(A_indices[0], dtype=np.int64)
    cols_all = np.asarray(A_indices[1], dtype=np.int64)
    vals_all = np.asarray(A_values, dtype=np.float32)

    # Per-rank sparse slices + row degrees.
    rank_data = []
    for r in range(NCORES):
        m = (rows_all // CLOC) == r
        rl = (rows_all[m] - r * CLOC).astype(np.int64)
        cc = cols_all[m]
        vv = vals_all[m]
        deg = np.bincount(rl, minlength=CLOC)
        rank_data.append((rl, cc, vv, deg))

    # ---- Round 1: assign each local class row to half A or half B,
    # balancing total degree (each half holds exactly CLOC/2 rows).
    half_of = []          # per core: row -> 0/1
    for r in range(NCORES):
        deg = rank_data[r][3]
        order = np.argsort(-deg, kind="stable")
        loads = np.zeros(2, dtype=np.int64)
        cnts = np.zeros(2, dtype=np.int64)
        hh = np.empty(CLOC, dtype=np.int64)
        for row in order:
            h = int(np.argmin(np.where(cnts < CLOC // 2, loads, np.iinfo(np.int64).max)))
            hh[row] = h
            loads[h] += deg[row]
            cnts[h] += 1
        half_of.append(hh)

    # Source-half of every contribution is now fixed (depends only on the
    # owner core's half assignment).  Compute per-row (degA, degB).
    degAB = []
    for r in range(NCORES):
        rl, cc, vv, _deg = rank_data[r]
        src_half = np.empty(len(cc), dtype=np.int64)
        for r2 in range(NCORES):
            m2 = (cc // CLOC) == r2
            src_half[m2] = half_of[r2][cc[m2] % CLOC]
        dA = np.bincount(rl[src_half == 0], minlength=CLOC)
        dB = np.bincount(rl[src_half == 1], minlength=CLOC)
        degAB.append((dA, dB, src_half))

    # ---- Round 2: within each half, pack rows into 32 blocks of 128 rows,
    # respecting a fixed per-block (capA, capB) chunk-capacity profile so the
    # SPMD chunk structure is identical on every core.  Fat blocks at the end
    # of each half absorb the tails.
    nfat = 4
    while True:
        capA = np.full(NBH, 2 * P, dtype=np.int64)
        capB = np.full(NBH, 2 * P, dtype=np.int64)
        capA[NBH - nfat:] = 3 * P
        capB[NBH - nfat:] = 3 * P
        perms = []
        ok = True
        for r in range(NCORES):
            dA, dB, _ = degAB[r]
            hh = half_of[r]
            perm = np.empty(CLOC, dtype=np.int64)
            for h in range(2):
                rows_h = np.where(hh == h)[0]
                order = np.argsort(-(dA[rows_h] + dB[rows_h]), kind="stable")
                loadsA = np.zeros(NBH, dtype=np.int64)
                loadsB = np.zeros(NBH, dtype=np.int64)
                cnts = np.zeros(NBH, dtype=np.int64)
                for row in rows_h[order]:
                    a, bb = dA[row], dB[row]
                    score = np.maximum((loadsA + a) / capA, (loadsB + bb) / capB)
                    score[cnts >= P] = np.inf
                    score[loadsA + a > capA] = np.inf
                    score[loadsB + bb > capB] = np.inf
                    blk = int(np.argmin(score))
                    if not np.isfinite(score[blk]):
                        ok = False
                        break
                    perm[row] = (h * NBH + blk) * P + cnts[blk]
                    loadsA[blk] += a
                    loadsB[blk] += bb
                    cnts[blk] += 1
                if not ok:
                    break
            if not ok:
                break
            perms.append(perm)
        if ok:
            break
        nfat += 4
        if nfat > NBH:
            raise RuntimeError("packing failed")
    chunks_a = tuple(int(capA[rb % NBH] // P) for rb in range(NB))
    chunks_b = tuple(int(capB[rb % NBH] // P) for rb in range(NB))
    maxch = max(a + b for a, b in zip(chunks_a, chunks_b))
    new2old = [np.argsort(p) for p in perms]

    cumt = np.array(CUMT, dtype=np.int64)
    tch = np.array(TCH, dtype=np.int64)
    ppfx = np.array(PASS_PREFIX[:-1], dtype=np.int64)

    # First sweep: per-core contribution arrays + per-chunk passes.
    core_arr = []
    chunk_pass = np.zeros((NCORES, NB, maxch), dtype=np.int64)
    for r in range(NCORES):
        rl, cc, vv, _deg = rank_data[r]
        rl_new = perms[r][rl]
        rr = cc // CLOC
        lnew = np.empty(len(cc), dtype=np.int64)
        for r2 in range(NCORES):
            m2 = rr == r2
            lnew[m2] = perms[r2][cc[m2] % CLOC]
        stile = lnew // P
        spass = (stile[:, None] >= ppfx[None, :]).sum(axis=1)
        k_src = np.searchsorted(cumt[1:], stile, side="right")
        ag_row = (NCORES * P * cumt[k_src]
                  + rr * P * tch[k_src]
                  + (lnew - P * cumt[k_src]))
        # index relative to the pass's gather window
        win_row = ag_row - np.where(spass >= NPASS_A, HALF_ROWS, 0)
        assert win_row.min() >= 0 and win_row.max() < HALF_ROWS

        blk = rl_new // P
        order2 = np.lexsort((spass, blk))
        blk = blk[order2]
        spass_s = spass[order2]
        rl_new_s = rl_new[order2]
        vv_s = vv[order2]
        win_row_s = win_row[order2]

        # Slot positions: window-A contributions fill the first chunks_a[blk]
        # chunks; window-B contributions start at the B region.
        is_b = (spass_s >= NPASS_A).astype(np.int64)
        ca = np.array(chunks_a)[blk]
        key = blk * 2 + is_b
        counts = np.bincount(key, minlength=2 * NB)
        starts = np.zeros(2 * NB, dtype=np.int64)
        starts[1:] = np.cumsum(counts)[:-1]
        pos_in_grp = np.arange(len(blk)) - starts[key]
        pos = np.where(is_b == 0, pos_in_grp, ca * P + pos_in_grp)
        ch_idx = pos // P
        p_idx = pos % P
        np.maximum.at(chunk_pass[r], (blk, ch_idx), spass_s)
        core_arr.append((blk, ch_idx, p_idx, rl_new_s, vv_s, win_row_s))

    # Merge pass assignment across cores (program structure must be SPMD).
    merged = chunk_pass.max(axis=0)
    passes = []
    for rb in range(NB):
        ps = []
        for ci in range(chunks_a[rb] + chunks_b[rb]):
            if ci < chunks_a[rb]:
                ps.append(min(NPASS_A - 1, int(merged[rb, ci])))
            else:
                ps.append(max(NPASS_A, int(merged[rb, ci])))
        passes.append(tuple(ps))
    passes = tuple(passes)
    profile = (chunks_a, chunks_b, passes)

    chunks = tuple(a + b for a, b in zip(chunks_a, chunks_b))
    order3 = sorted(
        (passes[rb][ci], rb, ci) for rb in range(NB) for ci in range(chunks[rb])
    )
    goff = np.zeros((NB, maxch), dtype=np.int64)
    for gi, (_, rb, ci) in enumerate(order3):
        goff[rb, ci] = gi
    tot_ch = len(order3)

    per_rank = []
    for r in range(NCORES):
        blk, ch_idx, p_idx, rl_new_s, vv_s, win_row_s = core_arr[r]
        gcol = goff[blk, ch_idx]
        selv_host = np.zeros((P, tot_ch), dtype=np.float32)
        seld_host = np.zeros((P, tot_ch), dtype=np.float32)
        selv_host[p_idx, gcol] = vv_s
        seld_host[p_idx, gcol] = (rl_new_s % P).astype(np.float32)
        idx_flat = np.zeros(tot_ch * P, dtype=np.int16)
        idx_flat[gcol * P + p_idx] = win_row_s.astype(np.int16)
        gidxs_host = np.ascontiguousarray(_wrap16(idx_flat))

        rows = slice(r * CLOC, (r + 1) * CLOC)
        wr = wt2_w[rows][new2old[r]]
        wt_host = np.ascontiguousarray(
            wr.reshape(NB, P, ND, P).transpose(0, 3, 2, 1)
        ).reshape(NB, P, D).astype(MM_NP)
        bias_host = np.ascontiguousarray(
            wt2_b[rows][new2old[r]].reshape(NB, P).T)
        per_rank.append({
            "enc": enc_flat,
            "gidx": gidx_host,
            "wt": wt_host,
            "biasv": bias_host,
            "selv": selv_host,
            "seld": seld_host,
            "iotav": np.tile(np.arange(P, dtype=np.float32), (P, 1)),
            "gidxs": gidxs_host,
        })
    return per_rank, profile, new2old


def kernel(**inputs) -> np.ndarray:
    per_rank, profile, new2old = _prep_host(
        inputs["enc_out"], inputs["wt2_w"], inputs["wt2_b"],
        inputs["A_values"], inputs["batch_idx"], inputs["tgt"],
        inputs["A_indices"])
    nc = _build_program(profile)
    res = None
    last_exc = None
    for _attempt in range(3):
        try:
            res = run_bass_kernel_spmd(
                nc, per_rank, core_ids=list(range(NCORES)), trace=TRACE)
            break
        except Exception as e:  # transient runtime/collective hiccups
            last_exc = e
    if res is None:
        raise last_exc
    global LAST_RESULTS
    LAST_RESULTS = res
    outT_full = np.empty((C, N), dtype=np.float32)
    for r in range(NCORES):
        outT_full[r * CLOC + new2old[r]] = res.results[r]["outT"].astype(np.float32)
    return np.ascontiguousarray(outT_full.T)


# revision 28
# speedup vs baseline: 1.0895x; 1.0257x over previous
"""Trainium2 Bass kernel for nn_Enet_81037442941606 (gnn_message_passing).

Computation (reference):
    g   = enc_out[batch_idx, tgt]                      # [N, D] gather
    h0  = batchnorm(g)  (training stats, biased var)   # [N, D]
    h1  = swish(h0 @ wt2_w.T + wt2_b)                  # [N, C]
    out = h1 @ A.T + h1   (A sparse, NNZ entries)      # [N, C]

Strategy (8 NeuronCores, tensor parallel over the class axis):
  * Each core owns a contiguous block of C/8 = 8192 classes: its wt2_w rows,
    its A rows (spmm output rows), and its output columns.
  * The h1^T shard exchange (bf16 AllGather, ~260us of CC ring time) is the
    critical resource.  It is split into 5 chunked collectives over class
    sub-ranges, each fired as soon as the main matmul finishes that
    sub-range, so the CC stream overlaps the matmul and most of the spmm.
  * The spmm runs in 4 passes keyed to AllGather-chunk completion: the host
    sorts each output row-block's contributions by source availability and
    packs them into 128-slot selection chunks; a chunk runs in the earliest
    pass whose ag_out prefix covers all its sources.  Partial psum results
    between passes are parked in SBUF (bf16) and merged back via identity
    matmuls.  Row gathers use the batched dma_gather instruction (int16
    indices, so sources address one of two 32768-row windows of ag_out;
    chunks are kept window-pure by the packing).
  * Host concatenates the 8 output shards and transposes back to [N, C].
"""

import numpy as np
import ml_dtypes

import concourse.bacc as bacc
import concourse.bass as bass
import concourse.mybir as mybir
import concourse.tile as tile
from concourse.bass_utils import run_bass_kernel_spmd
from concourse.masks import make_identity

# Problem sizes (hardcoded per contest rules).
B, S, D, C, N = 32, 128, 1024, 65536, 512
NNZ = 262144
EPS = 1e-5
NCORES = 8
CLOC = C // NCORES          # classes per core = 8192
NB = CLOC // 128            # 64 c-tiles (row-blocks) per core
NBH = NB // 2               # 32 c-tiles per half
ND = D // 128               # 8 contraction chunks
NT = N // 128               # 4 token tiles
P = 128

EX_DT = mybir.dt.bfloat16   # h1 exchange dtype
EX_NP = ml_dtypes.bfloat16
MM_DT = mybir.dt.bfloat16   # main-matmul operand dtype (W, h0^T)
MM_NP = ml_dtypes.bfloat16

# AllGather chunking (c-tiles per chunk) and spmm pass prefixes (c-tiles).
TCH = (8, 24, 24, 8)
CUMT = (0, 8, 32, 56, 64)
NAG = len(TCH)
PASS_PREFIX = (8, 32, 56, 64)       # pass j sources lie in tiles < this
NPASS = len(PASS_PREFIX)
NPASS_A = 2                         # passes sourcing window A (prefix <= 32)
HALF_ROWS = NBH * NCORES * P        # 32768: dma_gather int16 window size
WB = 4                      # W tiles per load group
GB = 4                      # sel-chunks per batched spmm gather
SB = 32                     # sel-chunks per sel-matrix DMA

_PROGRAM_CACHE = {}
TRACE = False          # set by test.py to capture an NTFF profile
LAST_RESULTS = None    # BassKernelResults of the last kernel() call


def _build_program(profile):
    """Build + compile the SPMD Bass program (identical on all 8 cores).

    profile = (chunks_a, chunks_b, passes):
      chunks_a[rb]/chunks_b[rb] = sel chunks sourcing window A (pass 0) /
      window B (passes 1-3) for row block rb; passes[rb] = per-chunk pass.
    """
    if profile in _PROGRAM_CACHE:
        return _PROGRAM_CACHE[profile]
    chunks_a, chunks_b, passes = profile
    chunks = tuple(a + b for a, b in zip(chunks_a, chunks_b))
    tot_ch = sum(chunks)

    # Global chunk order: (pass, rb, chunk_idx).
    order = sorted(
        (passes[rb][ci], rb, ci) for rb in range(NB) for ci in range(chunks[rb])
    )
    gidx_of = {(rb, ci): gi for gi, (_, rb, ci) in enumerate(order)}
    sessions = [[] for _ in range(NPASS)]
    first_pass = {}
    last_pass = {}
    for rb in range(NB):
        ps = passes[rb]
        first_pass[rb] = ps[0]
        last_pass[rb] = ps[-1]
        for j in sorted(set(ps)):
            cis = [ci for ci in range(chunks[rb]) if ps[ci] == j]
            gi0 = gidx_of[(rb, cis[0])]
            sessions[j].append((rb, gi0, gi0 + len(cis)))
    for j in range(NPASS):
        sessions[j].sort(key=lambda s: s[1])
    pass_start = [min((s[1] for s in sessions[j]), default=0)
                  for j in range(NPASS)]

    nc = bacc.Bacc("TRN2", target_bir_lowering=False, debug=False,
                   num_devices=NCORES)
    f32 = mybir.dt.float32
    i16 = mybir.dt.int16

    enc = nc.dram_tensor("enc", [B * S, D], f32, kind="ExternalInput")
    gidx = nc.dram_tensor("gidx", [P, NT * P // 16], i16, kind="ExternalInput")
    wt = nc.dram_tensor("wt", [NB, P, D], MM_DT, kind="ExternalInput")
    biasv = nc.dram_tensor("biasv", [P, NB], f32, kind="ExternalInput")
    selv = nc.dram_tensor("selv", [P, tot_ch], f32, kind="ExternalInput")
    seld = nc.dram_tensor("seld", [P, tot_ch], f32, kind="ExternalInput")
    iotav = nc.dram_tensor("iotav", [P, GB * P], f32, kind="ExternalInput")
    gidxs = nc.dram_tensor("gidxs", [P, tot_ch * 8], i16, kind="ExternalInput")
    outT = nc.dram_tensor("outT", [CLOC, N], EX_DT, kind="ExternalOutput")

    ag_ins = [nc.dram_tensor(f"ag_in{k}", [TCH[k] * P, N], EX_DT)
              for k in range(NAG)]
    ag_out = nc.dram_tensor("ag_out", [C, N], EX_DT, addr_space="Shared")
    # ag_out row space is chunk-major: chunk k rows live at
    # 8*P*CUMT[k] + rr*P*TCH[k] + (l - P*CUMT[k]).  Host maps gather
    # indices to this layout, relative to the pass's window base.
    ag_in_vs = [t.ap().rearrange("(i p) n -> p i n", p=P) for t in ag_ins]
    outT_v = outT.ap().rearrange("(i p) n -> i p n", p=P)

    with tile.TileContext(nc) as tc:
        with (
            tc.tile_pool(name="persist", bufs=1) as persist,
        ):
            h0T = persist.tile([P, ND * N], MM_DT)      # [d%128, (dchunk, n)]
            h1T = persist.tile([P, NB * N], EX_DT)      # [c%128, (ctile, n)]
            partial = persist.tile([P, NB * N], EX_DT)  # spmm pass partials
            bias_t = persist.tile([P, NB], f32)
            gidxs_t = persist.tile([P, tot_ch * 8], i16)
            selv_t = persist.tile([P, tot_ch], f32)
            seld_t = persist.tile([P, tot_ch], f32)
            iota_t = persist.tile([P, GB * P], f32)
            ident = persist.tile([P, P], f32)
            identb = persist.tile([P, P], EX_DT)
            mean_s = persist.tile([P, ND], f32)
            rstd_s = persist.tile([P, ND], f32)
            gidx_t = persist.tile([P, NT * P // 16], i16)

            nc.sync.dma_start(out=gidx_t[:], in_=gidx[:])
            make_identity(nc, ident[:])
            nc.vector.tensor_copy(out=identb[:], in_=ident[:])
            nc.sync.dma_start(out=bias_t[:], in_=biasv[:])
            nc.sync.dma_start(out=gidxs_t[:], in_=gidxs[:])
            nc.sync.dma_start(out=selv_t[:], in_=selv[:])
            nc.sync.dma_start(out=seld_t[:], in_=seld[:])
            nc.sync.dma_start(out=iota_t[:], in_=iotav[:])

            # ---------------- Phase A: gather + batchnorm + h0^T -----------
            with (
                tc.tile_pool(name="phA", bufs=1) as phA,
                tc.tile_pool(name="psA", bufs=4, space="PSUM") as psA,
            ):
                g_all = phA.tile([P, NT * D], f32, tag="g")
                nc.gpsimd.dma_gather(
                    out_ap=g_all[:].rearrange("p (b e) -> p b e", e=D),
                    in_ap=enc[:],
                    idxs_ap=gidx_t[:],
                    num_idxs=NT * P,
                    num_idxs_reg=NT * P,
                    elem_size=D,
                )

                # Raw transpose g -> h0T (tokens on the free axis), d-chunk
                # major so per-chunk batch stats chase the transposes.
                sum_s = phA.tile([P, ND], f32, tag="sums")
                sq_s = phA.tile([P, ND], f32, tag="sqs")
                scr = phA.tile([P, N], f32, tag="scr")
                for i in range(ND):
                    for j in range(NT):
                        tp = psA.tile([P, P], f32, space="PSUM", tag="tp")
                        nc.tensor.transpose(
                            tp[:], g_all[:, j * D + i * P: j * D + (i + 1) * P],
                            ident[:])
                        nc.vector.tensor_copy(
                            out=h0T[:, i * N + j * P: i * N + (j + 1) * P],
                            in_=tp[:])
                    nc.scalar.activation(
                        scr[:], h0T[:, i * N:(i + 1) * N],
                        mybir.ActivationFunctionType.Copy,
                        accum_out=sum_s[:, i:i + 1])
                    nc.scalar.activation(
                        scr[:], h0T[:, i * N:(i + 1) * N],
                        mybir.ActivationFunctionType.Square,
                        accum_out=sq_s[:, i:i + 1])

                ex2_s = phA.tile([P, ND], f32, tag="ex2")
                var_s = phA.tile([P, ND], f32, tag="var")
                nc.scalar.mul(mean_s[:], sum_s[:], 1.0 / N)
                nc.scalar.mul(ex2_s[:], sq_s[:], 1.0 / N)
                nc.vector.tensor_tensor(
                    out=var_s[:], in0=mean_s[:], in1=mean_s[:],
                    op=mybir.AluOpType.mult)
                nc.vector.tensor_tensor(
                    out=var_s[:], in0=ex2_s[:], in1=var_s[:],
                    op=mybir.AluOpType.subtract)
                sd_s = phA.tile([P, ND], f32, tag="sd")
                epsb = phA.tile([P, 1], f32, tag="epsb")
                nc.vector.memset(epsb[:], EPS)
                nc.scalar.activation(
                    sd_s[:], var_s[:], mybir.ActivationFunctionType.Sqrt,
                    bias=epsb[:, :1], scale=1.0)
                nc.vector.reciprocal(rstd_s[:], sd_s[:])

                for i in range(ND):
                    nc.vector.tensor_scalar(
                        out=h0T[:, i * N:(i + 1) * N],
                        in0=h0T[:, i * N:(i + 1) * N],
                        scalar1=mean_s[:, i:i + 1],
                        scalar2=rstd_s[:, i:i + 1],
                        op0=mybir.AluOpType.subtract,
                        op1=mybir.AluOpType.mult,
                    )

                # PE p-state warmup: back-to-back dummy matmuls so phase B
                # starts at full clock (the ramp needs >3us continuous busy).
                wps = psA.tile([P, P], f32, space="PSUM", tag="tp")
                for _ in range(36):
                    nc.tensor.matmul(out=wps[:], lhsT=identb[:],
                                     rhs=identb[:], start=True, stop=True)

            # ---------------- Phases B + D interleaved ---------------------
            # B: h1^T = swish(W h0^T + b), in AG-chunk order; each chunked
            # AllGather fires as soon as its c-tile range is in ag_in[k].
            # D: spmm passes are emitted between B chunks so the gpsimd
            # engine can start pass-j gathers as soon as AG chunk j lands,
            # without blocking later AG triggers (in-order engines).
            # Finals (which read h1T residuals) only occur in passes >= 2,
            # after all of B has been emitted.
            assert all(ps[-1] >= NPASS_A for ps in passes)
            wt_b = wt.ap().rearrange("(a b) p d -> a b p d", b=WB)
            with (
                tc.tile_pool(name="phB", bufs=2) as phB,
                tc.tile_pool(name="psB", bufs=4, space="PSUM") as psB,
                tc.tile_pool(name="ctp", bufs=7) as ctp,
                tc.tile_pool(name="selp", bufs=4) as selp,
                tc.tile_pool(name="otp", bufs=3) as otp,
                tc.tile_pool(name="psD", bufs=4, space="PSUM") as psD,
            ):
                ct_tiles = {}
                sel_tiles = {}

                def emit_b_chunk(k):
                    for a in range(CUMT[k] // WB, CUMT[k + 1] // WB):
                        wt_a = phB.tile([P, WB * D], MM_DT, tag="wt")
                        nc.sync.dma_start(
                            out=wt_a[:].rearrange("p (b d) -> p b d", b=WB),
                            in_=wt_b[a].rearrange("b p d -> p b d"))
                        for bsub in range(WB):
                            i = a * WB + bsub
                            h1ps = psB.tile([P, N], f32, space="PSUM",
                                            tag="h1ps")
                            for kk in range(ND):
                                nc.tensor.matmul(
                                    out=h1ps[:],
                                    lhsT=wt_a[:, bsub * D + kk * P:
                                              bsub * D + (kk + 1) * P],
                                    rhs=h0T[:, kk * N:(kk + 1) * N],
                                    start=(kk == 0), stop=(kk == ND - 1),
                                )
                            nc.scalar.activation(
                                h1T[:, i * N:(i + 1) * N], h1ps[:],
                                mybir.ActivationFunctionType.Silu,
                                bias=bias_t[:, i:i + 1], scale=1.0)
                        g0 = a * WB
                        nc.sync.dma_start(
                            out=ag_in_vs[k][:, g0 - CUMT[k]:
                                            g0 - CUMT[k] + WB, :],
                            in_=h1T[:, g0 * N:(g0 + WB) * N].rearrange(
                                "p (i n) -> p i n", n=N))
                    r0, r1 = NCORES * P * CUMT[k], NCORES * P * CUMT[k + 1]
                    nc.gpsimd.collective_compute(
                        "AllGather",
                        mybir.AluOpType.bypass,
                        replica_groups=[list(range(NCORES))],
                        ins=[ag_ins[k][:].opt()],
                        outs=[ag_out[r0:r1, :].opt()],
                    )

                def ensure_ct(j, b, pend):
                    key = (j, b)
                    if key in ct_tiles:
                        return ct_tiles[key]
                    a0 = pass_start[j] + b * GB
                    a1 = min(a0 + GB, pend)
                    m = a1 - a0
                    t = ctp.tile([P, GB * N], EX_DT, tag="ct")
                    base = 0 if j < NPASS_A else HALF_ROWS
                    ext = NCORES * P * PASS_PREFIX[j] - base
                    nc.gpsimd.dma_gather(
                        out_ap=t[:, :m * N].rearrange("p (b n) -> p b n", n=N),
                        in_ap=ag_out[base:base + ext, :],
                        idxs_ap=gidxs_t[:, a0 * 8:a1 * 8],
                        num_idxs=m * P,
                        num_idxs_reg=m * P,
                        elem_size=N,
                    )
                    # batched sel expansion for the same chunk range:
                    # sel[p, b, c] = (c == dst[p, b]) * val[p, b]
                    mk = selp.tile([P, GB * P], f32, tag="mask")
                    sl = selp.tile([P, GB * P], EX_DT, tag="sel")
                    nc.vector.tensor_tensor(
                        out=mk[:, :m * P].rearrange("p (b c) -> p b c", c=P),
                        in0=iota_t[:, :m * P].rearrange("p (b c) -> p b c", c=P),
                        in1=seld_t[:, a0:a1].to_broadcast([P, m, P]),
                        op=mybir.AluOpType.is_equal)
                    nc.vector.tensor_tensor(
                        out=sl[:, :m * P].rearrange("p (b c) -> p b c", c=P),
                        in0=mk[:, :m * P].rearrange("p (b c) -> p b c", c=P),
                        in1=selv_t[:, a0:a1].to_broadcast([P, m, P]),
                        op=mybir.AluOpType.mult)
                    ct_tiles[key] = (t, sl)
                    return t

                def emit_d_pass(j):
                    if not sessions[j]:
                        return
                    pend = max(s[2] for s in sessions[j])
                    for rb, gi0, gi1 in sessions[j]:
                        has_prev = j > first_pass[rb]
                        is_last = j == last_pass[rb]
                        acc = psD.tile([P, N], f32, space="PSUM", tag="acc")
                        for gi in range(gi0, gi1):
                            ctb = (gi - pass_start[j]) // GB
                            cto = gi - pass_start[j] - ctb * GB
                            ensure_ct(j, ctb, pend)
                            ct, sl4 = ct_tiles[(j, ctb)]
                            last_mm = ((gi == gi1 - 1) and not has_prev
                                       and not is_last)
                            nc.tensor.matmul(
                                out=acc[:],
                                lhsT=sl4[:, cto * P:(cto + 1) * P],
                                rhs=ct[:, cto * N:(cto + 1) * N],
                                start=(gi == gi0), stop=last_mm,
                            )
                        if has_prev:
                            nc.tensor.matmul(
                                out=acc[:],
                                lhsT=identb[:],
                                rhs=partial[:, rb * N:(rb + 1) * N],
                                start=False, stop=not is_last,
                            )
                        if is_last:
                            nc.tensor.matmul(
                                out=acc[:],
                                lhsT=identb[:],
                                rhs=h1T[:, rb * N:(rb + 1) * N],
                                start=False, stop=True,
                            )
                            o_t = otp.tile([P, N], EX_DT, tag="ot")
                            nc.scalar.activation(
                                o_t[:], acc[:],
                                mybir.ActivationFunctionType.Copy)
                            nc.sync.dma_start(out=outT_v[rb], in_=o_t[:])
                        else:
                            nc.scalar.activation(
                                partial[:, rb * N:(rb + 1) * N], acc[:],
                                mybir.ActivationFunctionType.Copy)

                for k in range(NAG):
                    emit_b_chunk(k)
                for j in range(NPASS):
                    emit_d_pass(j)

    nc.compile()
    _PROGRAM_CACHE[profile] = nc
    return nc


def _wrap16(idx_flat):
    """Layout flat gather indices for dma_gather: idx i -> [i%16, i//16],
    replicated across the 8 groups of 16 partitions."""
    n = len(idx_flat)
    assert n % 16 == 0
    a = np.asarray(idx_flat, dtype=np.int16).reshape(n // 16, 16).T  # [16, n/16]
    return np.tile(a, (8, 1))                                        # [128, n/16]


def _prep_host(enc_out, wt2_w, wt2_b, A_values, batch_idx, tgt, A_indices):
    """Shard inputs + restructure the sparse matrix for the device program."""
    enc_flat = np.ascontiguousarray(
        np.asarray(enc_out, dtype=np.float32).reshape(B * S, D))
    flat_idx = (np.asarray(batch_idx, dtype=np.int64) * S
                + np.asarray(tgt, dtype=np.int64))
    gidx_host = np.ascontiguousarray(_wrap16(flat_idx))

    wt2_w = np.asarray(wt2_w, dtype=np.float32)
    wt2_b = np.asarray(wt2_b, dtype=np.float32)
    rows_all = np.asarray(A_indices[0], dtype=np.int64)
    cols_all = np.asarray(A_indices[1], dtype=np.int64)
    vals_all = np.asarray(A_values, dtype=np.float32)

    # Per-rank sparse slices + row degrees.
    rank_data = []
    for r in range(NCORES):
        m = (rows_all // CLOC) == r
        rl = (rows_all[m] - r * CLOC).astype(np.int64)
        cc = cols_all[m]
        vv = vals_all[m]
        deg = np.bincount(rl, minlength=CLOC)
        rank_data.append((rl, cc, vv, deg))

    # ---- Round 1: assign each local class row to half A or half B,
    # balancing total degree (each half holds exactly CLOC/2 rows).
    half_of = []          # per core: row -> 0/1
    for r in range(NCORES):
        deg = rank_data[r][3]
        order = np.argsort(-deg, kind="stable")
        loads = np.zeros(2, dtype=np.int64)
        cnts = np.zeros(2, dtype=np.int64)
        hh = np.empty(CLOC, dtype=np.int64)
        for row in order:
            h = int(np.argmin(np.where(cnts < CLOC // 2, loads, np.iinfo(np.int64).max)))
            hh[row] = h
            loads[h] += deg[row]
            cnts[h] += 1
        half_of.append(hh)

    # Source-half of every contribution is now fixed (depends only on the
    # owner core's half assignment).  Compute per-row (degA, degB).
    degAB = []
    for r in range(NCORES):
        rl, cc, vv, _deg = rank_data[r]
        src_half = np.empty(len(cc), dtype=np.int64)
        for r2 in range(NCORES):
            m2 = (cc // CLOC) == r2
            src_half[m2] = half_of[r2][cc[m2] % CLOC]
        dA = np.bincount(rl[src_half == 0], minlength=CLOC)
        dB = np.bincount(rl[src_half == 1], minlength=CLOC)
        degAB.append((dA, dB, src_half))

    # ---- Round 2: within each half, pack rows into 32 blocks of 128 rows,
    # respecting a fixed per-block (capA, capB) chunk-capacity profile so the
    # SPMD chunk structure is identical on every core.  Fat blocks at the end
    # of each half absorb the tails.
    nfat = 4
    while True:
        capA = np.full(NBH, 2 * P, dtype=np.int64)
        capB = np.full(NBH, 2 * P, dtype=np.int64)
        capA[NBH - nfat:] = 3 * P
        capB[NBH - nfat:] = 3 * P
        perms = []
        ok = True
        for r in range(NCORES):
            dA, dB, _ = degAB[r]
            hh = half_of[r]
            perm = np.empty(CLOC, dtype=np.int64)
            for h in range(2):
                rows_h = np.where(hh == h)[0]
                order = np.argsort(-(dA[rows_h] + dB[rows_h]), kind="stable")
                loadsA = np.zeros(NBH, dtype=np.int64)
                loadsB = np.zeros(NBH, dtype=np.int64)
                cnts = np.zeros(NBH, dtype=np.int64)
                for row in rows_h[order]:
                    a, bb = dA[row], dB[row]
                    score = np.maximum((loadsA + a) / capA, (loadsB + bb) / capB)
                    score[cnts >= P] = np.inf
                    score[loadsA + a > capA] = np.inf
                    score[loadsB + bb > capB] = np.inf
                    blk = int(np.argmin(score))
                    if not np.isfinite(score[blk]):
                        ok = False
                        break
                    perm[row] = (h * NBH + blk) * P + cnts[blk]
                    loadsA[blk] += a
                    loadsB[blk] += bb
                    cnts[blk] += 1
                if not ok:
                    break
            if not ok:
                break
            perms.append(perm)
        if ok:
            break
        nfat += 4
        if nfat > NBH:
            raise RuntimeError("packing failed")
    chunks_a = tuple(int(capA[rb % NBH] // P) for rb in range(NB))
    chunks_b = tuple(int(capB[rb % NBH] // P) for rb in range(NB))
    maxch = max(a + b for a, b in zip(chunks_a, chunks_b))
    new2old = [np.argsort(p) for p in perms]

    cumt = np.array(CUMT, dtype=np.int64)
    tch = np.array(TCH, dtype=np.int64)
    ppfx = np.array(PASS_PREFIX[:-1], dtype=np.int64)

    # First sweep: per-core contribution arrays + per-chunk passes.
    core_arr = []
    chunk_pass = np.zeros((NCORES, NB, maxch), dtype=np.int64)
    for r in range(NCORES):
        rl, cc, vv, _deg = rank_data[r]
        rl_new = perms[r][rl]
        rr = cc // CLOC
        lnew = np.empty(len(cc), dtype=np.int64)
        for r2 in range(NCORES):
            m2 = rr == r2
            lnew[m2] = perms[r2][cc[m2] % CLOC]
        stile = lnew // P
        spass = (stile[:, None] >= ppfx[None, :]).sum(axis=1)
        k_src = np.searchsorted(cumt[1:], stile, side="right")
        ag_row = (NCORES * P * cumt[k_src]
                  + rr * P * tch[k_src]
                  + (lnew - P * cumt[k_src]))
        # index relative to the pass's gather window
        win_row = ag_row - np.where(spass >= NPASS_A, HALF_ROWS, 0)
        assert win_row.min() >= 0 and win_row.max() < HALF_ROWS

        blk = rl_new // P
        order2 = np.lexsort((spass, blk))
        blk = blk[order2]
        spass_s = spass[order2]
        rl_new_s = rl_new[order2]
        vv_s = vv[order2]
        win_row_s = win_row[order2]

        # Slot positions: window-A contributions fill the first chunks_a[blk]
        # chunks; window-B contributions start at the B region.
        is_b = (spass_s >= NPASS_A).astype(np.int64)
        ca = np.array(chunks_a)[blk]
        key = blk * 2 + is_b
        counts = np.bincount(key, minlength=2 * NB)
        starts = np.zeros(2 * NB, dtype=np.int64)
        starts[1:] = np.cumsum(counts)[:-1]
        pos_in_grp = np.arange(len(blk)) - starts[key]
        pos = np.where(is_b == 0, pos_in_grp, ca * P + pos_in_grp)
        ch_idx = pos // P
        p_idx = pos % P
        np.maximum.at(chunk_pass[r], (blk, ch_idx), spass_s)
        core_arr.append((blk, ch_idx, p_idx, rl_new_s, vv_s, win_row_s))

    # Merge pass assignment across cores (program structure must be SPMD).
    merged = chunk_pass.max(axis=0)
    passes = []
    for rb in range(NB):
        ps = []
        for ci in range(chunks_a[rb] + chunks_b[rb]):
            if ci < chunks_a[rb]:
                ps.append(min(NPASS_A - 1, int(merged[rb, ci])))
            else:
                ps.append(max(NPASS_A, int(merged[rb, ci])))
        passes.append(tuple(ps))
    passes = tuple(passes)
    profile = (chunks_a, chunks_b, passes)

    chunks = tuple(a + b for a, b in zip(chunks_a, chunks_b))
    order3 = sorted(
        (passes[rb][ci], rb, ci) for rb in range(NB) for ci in range(chunks[rb])
    )
    goff = np.zeros((NB, maxch), dtype=np.int64)
    for gi, (_, rb, ci) in enumerate(order3):
        goff[rb, ci] = gi
    tot_ch = len(order3)

    per_rank = []
    for r in range(NCORES):
        blk, ch_idx, p_idx, rl_new_s, vv_s, win_row_s = core_arr[r]
        gcol = goff[blk, ch_idx]
        selv_host = np.zeros((P, tot_ch), dtype=np.float32)
        seld_host = np.zeros((P, tot_ch), dtype=np.float32)
        selv_host[p_idx, gcol] = vv_s
        seld_host[p_idx, gcol] = (rl_new_s % P).astype(np.float32)
        idx_flat = np.zeros(tot_ch * P, dtype=np.int16)
        idx_flat[gcol * P + p_idx] = win_row_s.astype(np.int16)
        gidxs_host = np.ascontiguousarray(_wrap16(idx_flat))

        rows = slice(r * CLOC, (r + 1) * CLOC)
        wr = wt2_w[rows][new2old[r]]
        wt_host = np.ascontiguousarray(
            wr.reshape(NB, P, ND, P).transpose(0, 3, 2, 1)
        ).reshape(NB, P, D).astype(MM_NP)
        bias_host = np.ascontiguousarray(
            wt2_b[rows][new2old[r]].reshape(NB, P).T)
        per_rank.append({
            "enc": enc_flat,
            "gidx": gidx_host,
            "wt": wt_host,
            "biasv": bias_host,
            "selv": selv_host,
            "seld": seld_host,
            "iotav": np.tile(np.arange(P, dtype=np.float32), (P, GB)),
            "gidxs": gidxs_host,
        })
    return per_rank, profile, new2old


def kernel(**inputs) -> np.ndarray:
    per_rank, profile, new2old = _prep_host(
        inputs["enc_out"], inputs["wt2_w"], inputs["wt2_b"],
        inputs["A_values"], inputs["batch_idx"], inputs["tgt"],
        inputs["A_indices"])
    nc = _build_program(profile)
    res = None
    last_exc = None
    for _attempt in range(3):
        try:
            res = run_bass_kernel_spmd(
                nc, per_rank, core_ids=list(range(NCORES)), trace=TRACE)
            break
        except Exception as e:  # transient runtime/collective hiccups
            last_exc = e
    if res is None:
        raise last_exc
    global LAST_RESULTS
    LAST_RESULTS = res
    outT_full = np.empty((C, N), dtype=np.float32)
    for r in range(NCORES):
        outT_full[r * CLOC + new2old[r]] = res.results[r]["outT"].astype(np.float32)
    return np.ascontiguousarray(outT_full.T)
